# revision 4
# baseline (speedup 1.0000x reference)
"""BEV histogram + 4x(conv3x3+BN+ReLU) + 3x maxpool on 8 trn2 cores, v2.

Sharding: core = 2*b + h computes output rows [64h, 64h+64) of batch b.
Device pipeline per core (all per-layer activations in one SBUF buffer each):

- L1 in fp8 (e4m3): BEV built+quantized on host as [73, 34*1026] (72 rows =
  18 e-rows x 4 ch + const-1 bias row). Conv via x-pair DoubleRow matmuls:
  even/odd output columns computed separately, 2 fp8 weight blocks (hi + lo
  residual) -> 8 DR matmuls per tile = 2N cycles (vs 3N f16).
- L2-4 f16, 3 matmuls/tile, bias as const-1 K row (L4: bias in ACT epilogue).
- Drain per psum chain: ACT relu-copy-even -> DVE TT max(tmp, psum-odd) ->
  fold-copy Q[64:128] (Pool engine, or DVE in latency-critical phases) ->
  DVE final TT max -> next-layer buffer (f16). relu commutes with max
  (max(relu(a), b) == relu(max(a, b)) since relu(a) >= 0); bias is already
  in psum. Finals are emitted one chain late to hide the fold latency from
  the in-order DVE queue.
- Edges (SAME pad at y-borders): per-core *weight data* variants with the
  out-of-range K-rows zeroed - zero device ops.
- Halos: batched SBUF->SBUF DMAs (chunks of 4 tiles) on SP/HWDGE.
"""
import sys
sys.path.insert(0, '/opt/trn_rl_repo')
import numpy as np
import ml_dtypes

PR = [0.0, -39.68, -3.0, 69.12, 39.68, 1.0]
W = 1024
H = 1024
B = 4
BN_EPS = 1e-5
F8 = ml_dtypes.float8_e4m3

_CACHE = {}


def _bin_points(points):
    pts = np.asarray(points, dtype=np.float32)
    xs = np.float32(W / (PR[3] - PR[0]))
    ys = np.float32(H / (PR[4] - PR[1]))
    half = np.float32((PR[4] - PR[1]) / 2)
    xp = (pts[:, 1] * xs).astype(np.int32)
    yp = ((pts[:, 2] + half) * ys).astype(np.int32)
    b = pts[:, 0].astype(np.int32)
    mask = (xp >= 0) & (xp < W) & (yp >= 0) & (yp < H)
    lin = (b * H + yp) * W + xp
    z = pts[:, 3]
    inten = pts[:, 4]
    n = B * H * W
    lv = lin[mask]
    cnt = np.bincount(lv, minlength=n).astype(np.float32)
    zmin = np.full(n, 10.0, np.float32)
    np.minimum.at(zmin, lv, z[mask])
    zmax = np.full(n, -10.0, np.float32)
    np.maximum.at(zmax, lv, z[mask])
    iv = np.zeros(n, np.float32)
    np.maximum.at(iv, lv, inten[mask])
    bev0 = np.where(cnt == 0, np.float32(1.0), cnt) / np.float32(50.0)
    grids = np.stack([bev0, zmin, zmax, iv], axis=0).reshape(4, B, H, W)
    return np.transpose(grids, (1, 0, 2, 3))


def _fold_weights(w, b, g, be, m, v):
    scale = np.asarray(g, np.float32) / np.sqrt(np.asarray(v, np.float32) + np.float32(BN_EPS))
    wf = np.asarray(w, np.float32) * scale[:, None, None, None]
    bf = (np.asarray(b, np.float32) - np.asarray(m, np.float32)) * scale + np.asarray(be, np.float32)
    return wf.astype(np.float32), bf.astype(np.float32)


def _q8(x):
    return np.asarray(x, np.float32).astype(F8).astype(np.float32)


def _build_l1_wt(wf, bf):
    """-> [73, 7*128] e4m3 blob: blocks [w0h,w1h,w2h,w0l,w1l,w2l,Z].
    m = (y%2)*64 + (y//2)*8 + o; bias (hi/lo) on const row of blocks 0/3."""
    whi = _q8(wf)
    wlo = _q8(wf - whi)
    bhi = _q8(bf)
    blo = _q8(bf - bhi)
    blob = np.zeros((73, 7, 128), np.float32)
    for hl, wq in ((0, whi), (1, wlo)):
        for dx in range(3):
            blk = hl * 3 + dx
            for y in range(16):
                m0 = (y % 2) * 64 + (y // 2) * 8
                for dy in range(3):
                    e = y + dy
                    # rows e*4+c ; cols m0+o
                    blob[e * 4:(e + 1) * 4, blk, m0:m0 + 8] = wq[:, :, dy, dx].T
    for y in range(16):
        m0 = (y % 2) * 64 + (y // 2) * 8
        blob[72, 0, m0:m0 + 8] = bhi
        blob[72, 3, m0:m0 + 8] = blo
    return blob.reshape(73, 7 * 128).astype(F8)


def _m_index(y, o, co, co_major):
    if co_major:
        return o * 2 + y
    return (y % 2) * 64 + (y // 2) * co + o


def _build_wt16(wf, bf, ci, co, yoff, eta, co_major, bias_row, variants):
    """-> [K, nvar*384] f16. variants: list of zsets (e-row lists, or 'all')."""
    K = eta * ci + (1 if bias_row else 0)
    main = np.zeros((K, 3, 128), np.float32)
    for dx in range(3):
        for y in range(yoff):
            for dy in range(3):
                e = y + dy
                for o in range(co):
                    m = _m_index(y, o, co, co_major)
                    main[e * ci:(e + 1) * ci, dx, m] = wf[o, :, dy, dx]
    if bias_row:
        for y in range(yoff):
            for o in range(co):
                main[K - 1, 0, _m_index(y, o, co, co_major)] = bf[o]
    blobs = []
    for zset in variants:
        v = main.copy()
        if zset == 'all':
            v[:] = 0.0
        else:
            for e in zset:
                v[e * ci:(e + 1) * ci] = 0.0
        blobs.append(v)
    out = np.concatenate(blobs, axis=1)  # [K, nvar*3, 128]
    return out.reshape(K, -1).astype(np.float16)


def _build_b8(grid_b, h):
    """grid_b [4, 1024, 1024] f32 -> [73, 34*1026] e4m3 (incl ones row)."""
    from numpy.lib.stride_tricks import sliding_window_view
    g0 = 512 * h - 15
    q = np.asarray(grid_b, np.float32).astype(F8).astype(np.float32)
    padded = np.zeros((4, 546, 1026), np.float32)
    lo = max(0, g0)
    hi = min(1024, g0 + 546)
    padded[:, lo - g0:hi - g0, 1:1025] = q[:, lo:hi, :]
    wins = sliding_window_view(padded, 18, axis=1)    # [4, 529, 1026, 18]
    wins = wins[:, 0:16 * 34:16]                      # [4, 34, 1026, 18]
    tiles = np.transpose(wins, (1, 3, 0, 2))          # [34, 18, 4, 1026]
    tiles = np.ascontiguousarray(tiles).reshape(34, 72, 1026)
    ones = np.ones((34, 1, 1026), np.float32)
    full = np.concatenate([tiles, ones], axis=1)      # [34, 73, 1026]
    return np.ascontiguousarray(np.transpose(full, (1, 0, 2))).reshape(73, 34 * 1026).astype(F8)


def _build_module(debug=False):
    import concourse.mybir as mybir
    from concourse.tile import TileContext
    from concourse import bacc
    from concourse.ap import AP

    f32 = mybir.dt.float32
    f16 = mybir.dt.float16
    f8 = mybir.dt.float8e4
    AL = mybir.AluOpType
    RELU = mybir.ActivationFunctionType.Relu
    DR = mybir.MatmulPerfMode.DoubleRow

    nc = bacc.Bacc()
    b8_d = nc.dram_tensor("b8", [73, 34 * 1026], f8, kind="ExternalInput")
    wt8_d = nc.dram_tensor("wt8", [73, 7 * 128], f8, kind="ExternalInput")
    wt2_d = nc.dram_tensor("wt2", [81, 4 * 384], f16, kind="ExternalInput")
    wt3_d = nc.dram_tensor("wt3", [97, 3 * 384], f16, kind="ExternalInput")
    wt4_d = nc.dram_tensor("wt4", [128, 3 * 384], f16, kind="ExternalInput")
    bias4_d = nc.dram_tensor("bias4", [128, 1], f32, kind="ExternalInput")
    ones_d = nc.dram_tensor("ones", [1, 34 * 514], f16, kind="ExternalInput")
    out_d = nc.dram_tensor("out", [64, 64, 128], f32, kind="ExternalOutput")
    dbg = {}
    if debug:
        dbg["l2b"] = nc.dram_tensor("dbg_l2b", [81, 34 * 514], f16, kind="ExternalOutput")
        dbg["l3b"] = nc.dram_tensor("dbg_l3b", [97, 33 * 258], f16, kind="ExternalOutput")
        dbg["l4b"] = nc.dram_tensor("dbg_l4b", [128, 32 * 130], f16, kind="ExternalOutput")

    def ap3(t, off, pn, d1s, d1n, d2s, d2n):
        a = t[:]
        return AP(a.tensor, a.offset + off, [[a.ap[0][0], pn], [d1s, d1n], [d2s, d2n]])

    def ap_at(t, p0, pn, off, dims):
        a = t[p0:p0 + pn, :]
        return AP(a.tensor, a.offset + off, [[a.ap[0][0], pn]] + dims)

    with TileContext(nc) as tc:
        with tc.tile_pool(name="const", bufs=1) as cp, \
             tc.tile_pool(name="bufs", bufs=1) as bp, \
             tc.tile_pool(name="work", bufs=4) as wp, \
             tc.tile_pool(name="psum", bufs=1, space="PSUM") as pp:

            wt8 = cp.tile([73, 7 * 128], f8, tag="wt8")
            wt2 = cp.tile([81, 4 * 384], f16, tag="wt2")
            wt3 = cp.tile([97, 3 * 384], f16, tag="wt3")
            wt4 = cp.tile([128, 3 * 384], f16, tag="wt4")
            bias4 = cp.tile([128, 1], f32, tag="bias4")
            nc.sync.dma_start(out=wt8[:], in_=wt8_d[:])

            B8 = bp.tile([73, 34 * 1026], f8, tag="B8", name="B8")
            L2B = bp.tile([81, 34 * 514], f16, tag="L2B", name="L2B")
            L3B = bp.tile([97, 33 * 258], f16, tag="L3B", name="L3B")
            L4B = bp.tile([128, 32 * 130], f16, tag="L4B", name="L4B")
            scr3 = bp.tile([64, 256], f16, tag="scr3", name="scr3")
            scr4 = bp.tile([64, 128], f16, tag="scr4", name="scr4")

            # ones rows (const-1 bias rhs row for L2/L3)
            nc.sync.dma_start(out=L2B[80:81, :], in_=ones_d[:, 0:34 * 514])
            nc.sync.dma_start(out=L3B[96:97, :], in_=ones_d[:, 0:33 * 258])

            # x-pad zero columns + never-written halo of l2 tile 33
            def pad_memsets(buf, pn, ntiles, w_):
                nc.gpsimd.memset(buf[0:pn, 0:1], 0.0)
                nc.gpsimd.memset(ap_at(buf, 0, pn, w_ - 1, [[w_, ntiles - 1], [1, 2]]), 0.0)
                nc.gpsimd.memset(buf[0:pn, ntiles * w_ - 1:ntiles * w_], 0.0)
            pad_memsets(L2B, 80, 34, 514)
            pad_memsets(L3B, 96, 33, 258)
            pad_memsets(L4B, 128, 32, 130)
            nc.gpsimd.memset(L2B[64:80, 33 * 514:34 * 514], 0.0)

            # preload RELU act table while input DMAs are in flight
            warm = cp.tile([1, 2], f16, tag="warm")
            nc.gpsimd.memset(warm[:].bitcast(f32), 0.0)
            nc.scalar.activation(out=warm[:], in_=warm[:], func=RELU, scale=1.0)

            # input chunks: first small for fast start, weights interleaved early
            chunk_bounds = [0, 2, 6, 12, 18, 24, 29, 34]

            def b8_chunk(ci_):
                c0, c1 = chunk_bounds[ci_], chunk_bounds[ci_ + 1]
                nc.sync.dma_start(out=B8[:, c0 * 1026:c1 * 1026],
                                  in_=b8_d[:, c0 * 1026:c1 * 1026])
            b8_chunk(0)
            b8_chunk(1)
            nc.sync.dma_start(out=wt2[:], in_=wt2_d[:])
            b8_chunk(2)
            for t_, d_ in ((wt3, wt3_d), (wt4, wt4_d), (bias4, bias4_d)):
                nc.sync.dma_start(out=t_[:], in_=d_[:])
            for ci_ in range(3, 7):
                b8_chunk(ci_)

            wb8 = wt8[:]

            def lw(i, j):
                return AP(wb8.tensor, wb8.offset + i * 128,
                          [[wb8.ap[0][0], 73], [(j - i) * 128, 2], [1, 128]])

            b8a = B8[:]

            def pairs(off):
                return AP(b8a.tensor, b8a.offset + off, [[b8a.ap[0][0], 73], [1, 2], [2, 256]])

            L1_PAIRS_E = [(lw(0, 1), 0), (lw(3, 4), 0), (lw(2, 6), 2), (lw(5, 6), 2)]
            L1_PAIRS_O = [(lw(6, 0), 0), (lw(6, 3), 0), (lw(1, 2), 2), (lw(4, 5), 2)]

            def l1_tile(t):
                # tiles 0,1 borrow L2's psum slot (L2 starts at t>=5): depth 3
                # in the latency-critical warmup phase
                if t < 2:
                    P = pp.tile([128, 1024], f32, tag="ps2", name="ps2", bufs=1)
                else:
                    P = pp.tile([128, 1024], f32, tag="ps1", name="ps1", bufs=2)
                for reg, plist in ((0, L1_PAIRS_E), (256, L1_PAIRS_O)):
                    for hx in range(2):
                        base = t * 1026 + 512 * hx
                        o0 = hx * 512 + reg
                        for k, (lhs, poff) in enumerate(plist):
                            nc.tensor.matmul(out=P[:, o0:o0 + 256], lhsT=lhs,
                                             rhs=pairs(base + poff),
                                             start=(k == 0), stop=(k == 3),
                                             perf_mode=DR)
                tmp = wp.tile([128, 512], f16, tag="tmp1", name="tmp1")
                Q = wp.tile([128, 512], f16, tag="q1", name="q1")
                Pt = P[:]
                evens = AP(Pt.tensor, Pt.offset, [[Pt.ap[0][0], 128], [512, 2], [1, 256]])
                odds = AP(Pt.tensor, Pt.offset + 256, [[Pt.ap[0][0], 128], [512, 2], [1, 256]])
                nc.scalar.activation(out=tmp[:].rearrange("p (b x) -> p b x", b=2),
                                     in_=evens, func=RELU, scale=1.0)
                nc.vector.tensor_tensor(out=Q[:].rearrange("p (b x) -> p b x", b=2),
                                        in0=tmp[:].rearrange("p (b x) -> p b x", b=2),
                                        in1=odds, op=AL.max)
                R = wp.tile([64, 512], f16, tag="r1", name="r1")
                if t < 24:
                    nc.vector.tensor_copy(out=R[:], in_=Q[64:128, :])
                else:
                    nc.gpsimd.tensor_copy(out=R[:], in_=Q[64:128, :])
                pend1.append((t, Q, R))
                if len(pend1) > 1:
                    l1_final()

            def l1_final():
                t, Q, R = pend1.pop(0)
                nc.vector.tensor_tensor(out=L2B[0:64, t * 514 + 1:t * 514 + 513],
                                        in0=Q[0:64, :], in1=R[:], op=AL.max)

            def halo12(d0, n):
                nc.sync.dma_start(
                    out=ap_at(L2B, 64, 16, d0 * 514 + 1, [[514, n], [1, 512]]),
                    in_=ap_at(L2B, 0, 16, (d0 + 1) * 514 + 1, [[514, n], [1, 512]]))

            # L2 variant selection: tile -> variant index in wt2 blob
            def l2_var(u):
                return {0: 1, 32: 2, 33: 3}.get(u, 0)

            def l2_chain(c):
                u = 2 * c
                P = pp.tile([128, 1024], f32, tag="ps2", name="ps2", bufs=1)
                for i in range(2):
                    vb = l2_var(u + i) * 384
                    for dx in range(3):
                        nc.tensor.matmul(
                            out=P[:, i * 512:(i + 1) * 512],
                            lhsT=wt2[:, vb + dx * 128:vb + (dx + 1) * 128],
                            rhs=L2B[0:81, (u + i) * 514 + dx:(u + i) * 514 + dx + 512],
                            start=(dx == 0), stop=(dx == 2))
                Pt = P[:]
                evens = AP(Pt.tensor, Pt.offset, [[Pt.ap[0][0], 128], [512, 2], [2, 256]])
                odds = AP(Pt.tensor, Pt.offset + 1, [[Pt.ap[0][0], 128], [512, 2], [2, 256]])
                tmp = wp.tile([128, 512], f16, tag="tmp2", name="tmp2")
                nc.scalar.activation(out=tmp[:].rearrange("p (b x) -> p b x", b=2),
                                     in_=evens, func=RELU, scale=1.0)
                Q = wp.tile([128, 512], f16, tag="q2", name="q2")
                nc.vector.tensor_tensor(out=Q[:].rearrange("p (b x) -> p b x", b=2),
                                        in0=tmp[:].rearrange("p (b x) -> p b x", b=2),
                                        in1=odds, op=AL.max)
                R = wp.tile([64, 512], f16, tag="r2", name="r2")
                if c < 6 or c >= 14:
                    nc.vector.tensor_copy(out=R[:], in_=Q[64:128, :])
                else:
                    nc.gpsimd.tensor_copy(out=R[:], in_=Q[64:128, :])
                pend2.append((c, Q, R))
                if len(pend2) > 1:
                    l2_final()

            def l2_final():
                c, Q, R = pend2.pop(0)
                u = 2 * c
                if c < 16:
                    nc.vector.tensor_tensor(
                        out=ap_at(L3B, 0, 64, u * 258 + 1, [[258, 2], [1, 256]]),
                        in0=Q[0:64, :].rearrange("p (b x) -> p b x", b=2),
                        in1=R[:].rearrange("p (b x) -> p b x", b=2), op=AL.max)
                else:
                    nc.vector.tensor_tensor(out=L3B[0:64, 32 * 258 + 1:32 * 258 + 257],
                                            in0=Q[0:64, 0:256], in1=R[:, 0:256], op=AL.max)
                    nc.vector.tensor_tensor(out=scr3[:], in0=Q[0:64, 256:512],
                                            in1=R[:, 256:512], op=AL.max)

            def halo23(d0, n):
                nc.sync.dma_start(
                    out=ap_at(L3B, 64, 32, d0 * 258 + 1, [[258, n], [1, 256]]),
                    in_=ap_at(L3B, 0, 32, (d0 + 1) * 258 + 1, [[258, n], [1, 256]]))

            def halo23s():
                nc.sync.dma_start(out=L3B[64:96, 32 * 258 + 1:32 * 258 + 257],
                                  in_=scr3[0:32, :])

            def l3_var(v):
                return {0: 1, 32: 2}.get(v, 0)

            def l3_chain(d):
                v = 2 * d
                single = (d == 16)
                n = 1 if single else 2
                if d >= 14:
                    Pf = pp.tile([128, 1024], f32, tag="ps2", name="ps2", bufs=1)
                    P = Pf[0:128, 0:512]
                else:
                    P = pp.tile([128, 512], f32, tag="ps34", name="ps34", bufs=2)
                for i in range(n):
                    vb = l3_var(v + i) * 384
                    for dx in range(3):
                        nc.tensor.matmul(
                            out=P[:, i * 256:(i + 1) * 256],
                            lhsT=wt3[:, vb + dx * 128:vb + (dx + 1) * 128],
                            rhs=L3B[0:97, (v + i) * 258 + dx:(v + i) * 258 + dx + 256],
                            start=(dx == 0), stop=(dx == 2))
                Pt = P[:]
                wq = 128 * n
                evens = AP(Pt.tensor, Pt.offset, [[Pt.ap[0][0], 128], [256, n], [2, 128]])
                odds = AP(Pt.tensor, Pt.offset + 1, [[Pt.ap[0][0], 128], [256, n], [2, 128]])
                tmp = wp.tile([128, 256], f16, tag="tmp3", name="tmp3")
                nc.scalar.activation(out=tmp[:, 0:wq].rearrange("p (b x) -> p b x", b=n),
                                     in_=evens, func=RELU, scale=1.0)
                Q = wp.tile([128, 256], f16, tag="q3", name="q3")
                nc.vector.tensor_tensor(out=Q[:, 0:wq].rearrange("p (b x) -> p b x", b=n),
                                        in0=tmp[:, 0:wq].rearrange("p (b x) -> p b x", b=n),
                                        in1=odds, op=AL.max)
                R = wp.tile([64, 256], f16, tag="r3", name="r3")
                if d >= 12:
                    nc.vector.tensor_copy(out=R[:, 0:wq], in_=Q[64:128, 0:wq])
                else:
                    nc.gpsimd.tensor_copy(out=R[:, 0:wq], in_=Q[64:128, 0:wq])
                pend3.append((d, Q, R))
                if len(pend3) > 1:
                    l3_final()

            def l3_final():
                d, Q, R = pend3.pop(0)
                v = 2 * d
                if d < 16:
                    nc.vector.tensor_tensor(
                        out=ap_at(L4B, 0, 64, v * 130 + 1, [[130, 2], [1, 128]]),
                        in0=Q[0:64, 0:256].rearrange("p (b x) -> p b x", b=2),
                        in1=R[:, 0:256].rearrange("p (b x) -> p b x", b=2), op=AL.max)
                else:
                    nc.vector.tensor_tensor(out=scr4[:], in0=Q[0:64, 0:128],
                                            in1=R[:, 0:128], op=AL.max)

            def halo34(d0, n):
                nc.sync.dma_start(
                    out=ap_at(L4B, 64, 64, d0 * 130 + 1, [[130, n], [1, 128]]),
                    in_=ap_at(L4B, 0, 64, (d0 + 1) * 130 + 1, [[130, n], [1, 128]]))

            def halo34s():
                nc.sync.dma_start(out=L4B[64:128, 31 * 130 + 1:31 * 130 + 129],
                                  in_=scr4[:])

            def l4_var(w_):
                return {0: 1, 31: 2}.get(w_, 0)

            def l4_chain(e):
                w_ = 4 * e
                if e >= 6:
                    P4full = pp.tile([128, 1024], f32, tag="ps1", name="ps1", bufs=2)
                    P = P4full[0:128, 0:512]
                else:
                    P4full = pp.tile([128, 512], f32, tag="ps34", name="ps34", bufs=2)
                    P = P4full[:]
                for i in range(4):
                    vb = l4_var(w_ + i) * 384
                    for dx in range(3):
                        nc.tensor.matmul(
                            out=P[:, i * 128:(i + 1) * 128],
                            lhsT=wt4[:, vb + dx * 128:vb + (dx + 1) * 128],
                            rhs=L4B[0:128, (w_ + i) * 130 + dx:(w_ + i) * 130 + dx + 128],
                            start=(dx == 0), stop=(dx == 2))
                stage = wp.tile([128, 512], f32, tag="stage", name="stage")
                nc.scalar.activation(out=stage[:], in_=P[:], func=RELU,
                                     bias=bias4[:], scale=1.0)
                sa = stage[:]
                oa = out_d[:]
                pitch = sa.ap[0][0]
                for rp in range(2):
                    in_ap = AP(sa.tensor, sa.offset + rp * pitch,
                               [[2 * pitch, 64], [128, 4], [1, 128]])
                    out_ap = AP(oa.tensor, 1024 * e + 128 * rp,
                                [[8192, 64], [256, 4], [1, 128]])
                    nc.sync.dma_start(out=out_ap, in_=in_ap)

            # ---- emission schedule ----
            # Rate-limited round-robin: at most one unit per layer per step so
            # the in-order PE queue always has other work between same-layer
            # chains (psum bufs=1), with slack lags so halo-DMA latency is
            # hidden.
            state = dict(l2c=0, h23=0, l3c=0, h34=0, l4c=0, h23s=False, h34s=False)
            stage_ref = [None]
            halo12_cov = [0]
            pend1, pend2, pend3 = [], [], []

            def fin2():
                return state['l2c'] - len(pend2)

            def fin3():
                return state['l3c'] - len(pend3)

            def pump(max_per_layer=1):
                done = 0
                # halo23 chunks m=1..15 (dst [2m-2, 2m), src tiles <= 2m
                # which need chain-m's final emitted: fin2 >= m+1)
                m = state['h23'] + 1
                if m <= 15 and fin2() >= m + 1:
                    halo23(2 * (m - 1), 2); state['h23'] += 1
                if not state['h23s'] and state['l2c'] == 17 and not pend2:
                    halo23(30, 2)
                    state['h23'] += 1
                    halo23s(); state['h23s'] = True
                # halo34 chunks n=1..15 (dst [2n-2,2n)), then dst 30 + scr
                nn = state['h34'] + 1
                if nn <= 15 and fin3() >= nn + 1:
                    halo34(2 * (nn - 1), 2); state['h34'] += 1
                if not state['h34s'] and state['l3c'] == 17 and not pend3:
                    halo34(30, 1)
                    state['h34'] += 1
                    halo34s(); state['h34s'] = True
                for _ in range(max_per_layer):
                    c = state['l2c']
                    if c < 16 and halo12_cov[0] >= min(2 * c + 2, 33):
                        l2_chain(c); state['l2c'] += 1; done += 1
                    elif c == 16 and halo12_cov[0] >= 33:
                        l2_chain(16); state['l2c'] += 1; done += 1
                if pend2 and state['l2c'] == 17:
                    l2_final()
                for _ in range(max_per_layer):
                    d = state['l3c']
                    if d < 16 and 2 * state['h23'] >= min(2 * d + 6, 32):
                        l3_chain(d); state['l3c'] += 1; done += 1
                    elif d == 16 and state['h23s'] and state['l2c'] == 17:
                        l3_chain(16); state['l3c'] += 1; done += 1
                if pend3 and state['l3c'] == 17:
                    l3_final()
                for _ in range(max_per_layer):
                    e = state['l4c']
                    cov34 = 2 * min(state['h34'], 15) + (2 if state['h34'] >= 16 else 0) \
                        + (1 if state['h34s'] else 0)
                    if e < 8 and cov34 >= min(4 * e + 6, 33):
                        l4_chain(e); state['l4c'] += 1; done += 1
                return done

            for t in range(34):
                l1_tile(t)
                # halo12 G=2 at odd t: dst [t-3, t-1), src tiles t-2..t-1
                # (final(t-1) emitted during l1_tile(t))
                if t >= 3 and t % 2 == 1:
                    halo12(t - 3, 2)
                    halo12_cov[0] = t - 1
                pump()
            while pend1:
                l1_final()
            halo12(32, 1)
            halo12_cov[0] = 33
            for _ in range(64):
                if state['l4c'] == 8:
                    break
                pump()
            assert state['l4c'] == 8, state
            assert not pend1 and not pend2 and not pend3

            if debug:
                nc.sync.dma_start(out=dbg["l2b"][:], in_=L2B[:])
                nc.sync.dma_start(out=dbg["l3b"][:], in_=L3B[:])
                nc.sync.dma_start(out=dbg["l4b"][:], in_=L4B[:])

    nc.finalize()
    return nc


def _prep_weights(inputs):
    params = [
        _fold_weights(inputs['w1'], inputs['b1'], inputs['g1'], inputs['be1'], inputs['m1'], inputs['v1']),
        _fold_weights(inputs['w2'], inputs['b2'], inputs['g2'], inputs['be2'], inputs['m2'], inputs['v2']),
        _fold_weights(inputs['w3'], inputs['b3'], inputs['g3'], inputs['be3'], inputs['m3'], inputs['v3']),
        _fold_weights(inputs['w4'], inputs['b4'], inputs['g4'], inputs['be4'], inputs['m4'], inputs['v4']),
    ]
    wt8 = _build_l1_wt(*params[0])
    # variants per h: [main, v_t0, v_t32, v_t33] etc (zsets depend on h)
    wt2 = {}
    wt3 = {}
    wt4 = {}
    for h in (0, 1):
        if h == 0:
            z2 = [[], [0, 1, 2, 3, 4, 5, 6], [], []]
            z3 = [[], [0, 1, 2], []]
            z4 = [[], [0], []]
        else:
            z2 = [[], [], [7, 8, 9], 'all']
            z3 = [[], [], [3, 4, 5]]
            z4 = [[], [], [3]]
        wt2[h] = _build_wt16(params[1][0], params[1][1], 8, 16, 8, 10, False, True, z2)
        wt3[h] = _build_wt16(params[2][0], params[2][1], 16, 32, 4, 6, False, True, z3)
        wt4[h] = _build_wt16(params[3][0], params[3][1], 32, 64, 2, 4, True, False, z4)
    bias4 = np.zeros((128, 1), np.float32)
    bf4 = params[3][1]
    for o in range(64):
        for y in range(2):
            bias4[o * 2 + y, 0] = bf4[o]
    ones = np.ones((1, 34 * 514), np.float16)
    return wt8, wt2, wt3, wt4, bias4, ones


def kernel(points, batch_size,
           w1, b1, g1, be1, m1, v1,
           w2, b2, g2, be2, m2, v2,
           w3, b3, g3, be3, m3, v3,
           w4, b4, g4, be4, m4, v4, **_kw):
    from concourse.bass_utils import run_bass_kernel_spmd

    grids = _bin_points(points)
    inputs = dict(w1=w1, b1=b1, g1=g1, be1=be1, m1=m1, v1=v1,
                  w2=w2, b2=b2, g2=g2, be2=be2, m2=m2, v2=v2,
                  w3=w3, b3=b3, g3=g3, be3=be3, m3=m3, v3=v3,
                  w4=w4, b4=b4, g4=g4, be4=be4, m4=m4, v4=v4)
    wt8, wt2, wt3, wt4, bias4, ones = _prep_weights(inputs)

    core_ids = list(range(8))
    in_maps = []
    for core in core_ids:
        b, h = core // 2, core % 2
        im = {
            "b8": _build_b8(grids[b], h),
            "wt8": wt8,
            "wt2": wt2[h],
            "wt3": wt3[h],
            "wt4": wt4[h],
            "bias4": bias4,
            "ones": ones,
        }
        in_maps.append(im)

    if "nc" not in _CACHE:
        _CACHE["nc"] = _build_module()
    nc = _CACHE["nc"]

    r = run_bass_kernel_spmd(nc, in_maps, core_ids=core_ids)

    out_full = np.zeros((B, 64, 128, 128), np.float32)
    for i, core in enumerate(core_ids):
        b, h = core // 2, core % 2
        out_full[b, :, 64 * h:64 * h + 64, :] = r.results[i]["out"]
    return out_full


# revision 5
# speedup vs baseline: 1.0368x; 1.0368x over previous
"""BEV histogram + 4x(conv3x3+BN+ReLU) + 3x maxpool on 8 trn2 cores, v2.

Sharding: core = 2*b + h computes output rows [64h, 64h+64) of batch b.
Device pipeline per core (all per-layer activations in one SBUF buffer each):

- L1 in fp8 (e4m3): BEV built+quantized on host as [73, 34*1026] (72 rows =
  18 e-rows x 4 ch + const-1 bias row). Conv via x-pair DoubleRow matmuls:
  even/odd output columns computed separately, 2 fp8 weight blocks (hi + lo
  residual) -> 8 DR matmuls per tile = 2N cycles (vs 3N f16).
- L2-4 f16, 3 matmuls/tile, bias as const-1 K row (L4: bias in ACT epilogue).
- Drain per psum chain: ACT relu-copy-even -> DVE TT max(tmp, psum-odd) ->
  fold-copy Q[64:128] (Pool engine, or DVE in latency-critical phases) ->
  DVE final TT max -> next-layer buffer (f16). relu commutes with max
  (max(relu(a), b) == relu(max(a, b)) since relu(a) >= 0); bias is already
  in psum. Finals are emitted one chain late to hide the fold latency from
  the in-order DVE queue.
- Edges (SAME pad at y-borders): per-core *weight data* variants with the
  out-of-range K-rows zeroed - zero device ops.
- Halos: batched SBUF->SBUF DMAs (chunks of 4 tiles) on SP/HWDGE.
"""
import sys
sys.path.insert(0, '/opt/trn_rl_repo')
import numpy as np
import ml_dtypes

PR = [0.0, -39.68, -3.0, 69.12, 39.68, 1.0]
W = 1024
H = 1024
B = 4
BN_EPS = 1e-5
F8 = ml_dtypes.float8_e4m3

_CACHE = {}


def _bin_points(points):
    pts = np.asarray(points, dtype=np.float32)
    xs = np.float32(W / (PR[3] - PR[0]))
    ys = np.float32(H / (PR[4] - PR[1]))
    half = np.float32((PR[4] - PR[1]) / 2)
    xp = (pts[:, 1] * xs).astype(np.int32)
    yp = ((pts[:, 2] + half) * ys).astype(np.int32)
    b = pts[:, 0].astype(np.int32)
    mask = (xp >= 0) & (xp < W) & (yp >= 0) & (yp < H)
    lin = (b * H + yp) * W + xp
    z = pts[:, 3]
    inten = pts[:, 4]
    n = B * H * W
    lv = lin[mask]
    cnt = np.bincount(lv, minlength=n).astype(np.float32)
    zmin = np.full(n, 10.0, np.float32)
    np.minimum.at(zmin, lv, z[mask])
    zmax = np.full(n, -10.0, np.float32)
    np.maximum.at(zmax, lv, z[mask])
    iv = np.zeros(n, np.float32)
    np.maximum.at(iv, lv, inten[mask])
    bev0 = np.where(cnt == 0, np.float32(1.0), cnt) / np.float32(50.0)
    grids = np.stack([bev0, zmin, zmax, iv], axis=0).reshape(4, B, H, W)
    return np.transpose(grids, (1, 0, 2, 3))


def _fold_weights(w, b, g, be, m, v):
    scale = np.asarray(g, np.float32) / np.sqrt(np.asarray(v, np.float32) + np.float32(BN_EPS))
    wf = np.asarray(w, np.float32) * scale[:, None, None, None]
    bf = (np.asarray(b, np.float32) - np.asarray(m, np.float32)) * scale + np.asarray(be, np.float32)
    return wf.astype(np.float32), bf.astype(np.float32)


def _q8(x):
    return np.asarray(x, np.float32).astype(F8).astype(np.float32)


def _build_l1_wt(wf, bf):
    """-> [73, 7*128] e4m3 blob: blocks [w0h,w1h,w2h,w0l,w1l,w2l,Z].
    m = (y%2)*64 + (y//2)*8 + o; bias (hi/lo) on const row of blocks 0/3."""
    whi = _q8(wf)
    wlo = _q8(wf - whi)
    bhi = _q8(bf)
    blo = _q8(bf - bhi)
    blob = np.zeros((73, 7, 128), np.float32)
    for hl, wq in ((0, whi), (1, wlo)):
        for dx in range(3):
            blk = hl * 3 + dx
            for y in range(16):
                m0 = (y % 2) * 64 + (y // 2) * 8
                for dy in range(3):
                    e = y + dy
                    # rows e*4+c ; cols m0+o
                    blob[e * 4:(e + 1) * 4, blk, m0:m0 + 8] = wq[:, :, dy, dx].T
    for y in range(16):
        m0 = (y % 2) * 64 + (y // 2) * 8
        blob[72, 0, m0:m0 + 8] = bhi
        blob[72, 3, m0:m0 + 8] = blo
    return blob.reshape(73, 7 * 128).astype(F8)


def _m_index(y, o, co, co_major):
    if co_major:
        return o * 2 + y
    return (y % 2) * 64 + (y // 2) * co + o


def _build_wt16(wf, bf, ci, co, yoff, eta, co_major, bias_row, variants):
    """-> [K, nvar*384] f16. variants: list of zsets (e-row lists, or 'all')."""
    K = eta * ci + (1 if bias_row else 0)
    main = np.zeros((K, 3, 128), np.float32)
    for dx in range(3):
        for y in range(yoff):
            for dy in range(3):
                e = y + dy
                for o in range(co):
                    m = _m_index(y, o, co, co_major)
                    main[e * ci:(e + 1) * ci, dx, m] = wf[o, :, dy, dx]
    if bias_row:
        for y in range(yoff):
            for o in range(co):
                main[K - 1, 0, _m_index(y, o, co, co_major)] = bf[o]
    blobs = []
    for zset in variants:
        v = main.copy()
        if zset == 'all':
            v[:] = 0.0
        else:
            for e in zset:
                v[e * ci:(e + 1) * ci] = 0.0
        blobs.append(v)
    out = np.concatenate(blobs, axis=1)  # [K, nvar*3, 128]
    return out.reshape(K, -1).astype(np.float16)


def _build_b8(grid_b, h):
    """grid_b [4, 1024, 1024] f32 -> [73, 34*1026] e4m3 (incl ones row)."""
    from numpy.lib.stride_tricks import sliding_window_view
    g0 = 512 * h - 15
    q = np.asarray(grid_b, np.float32).astype(F8).astype(np.float32)
    padded = np.zeros((4, 546, 1026), np.float32)
    lo = max(0, g0)
    hi = min(1024, g0 + 546)
    padded[:, lo - g0:hi - g0, 1:1025] = q[:, lo:hi, :]
    wins = sliding_window_view(padded, 18, axis=1)    # [4, 529, 1026, 18]
    wins = wins[:, 0:16 * 34:16]                      # [4, 34, 1026, 18]
    tiles = np.transpose(wins, (1, 3, 0, 2))          # [34, 18, 4, 1026]
    tiles = np.ascontiguousarray(tiles).reshape(34, 72, 1026)
    ones = np.ones((34, 1, 1026), np.float32)
    full = np.concatenate([tiles, ones], axis=1)      # [34, 73, 1026]
    return np.ascontiguousarray(np.transpose(full, (1, 0, 2))).reshape(73, 34 * 1026).astype(F8)


def _build_module(debug=False):
    import concourse.mybir as mybir
    from concourse.tile import TileContext
    from concourse import bacc
    from concourse.ap import AP

    f32 = mybir.dt.float32
    f16 = mybir.dt.float16
    f8 = mybir.dt.float8e4
    AL = mybir.AluOpType
    RELU = mybir.ActivationFunctionType.Relu
    DR = mybir.MatmulPerfMode.DoubleRow

    nc = bacc.Bacc()
    b8_d = nc.dram_tensor("b8", [73, 34 * 1026], f8, kind="ExternalInput")
    wt8_d = nc.dram_tensor("wt8", [73, 7 * 128], f8, kind="ExternalInput")
    wt2_d = nc.dram_tensor("wt2", [81, 4 * 384], f16, kind="ExternalInput")
    wt3_d = nc.dram_tensor("wt3", [97, 3 * 384], f16, kind="ExternalInput")
    wt4_d = nc.dram_tensor("wt4", [128, 3 * 384], f16, kind="ExternalInput")
    bias4_d = nc.dram_tensor("bias4", [128, 1], f32, kind="ExternalInput")
    ones_d = nc.dram_tensor("ones", [1, 34 * 514], f16, kind="ExternalInput")
    out_d = nc.dram_tensor("out", [64, 64, 128], f32, kind="ExternalOutput")
    dbg = {}
    if debug:
        dbg["l2b"] = nc.dram_tensor("dbg_l2b", [81, 34 * 514], f16, kind="ExternalOutput")
        dbg["l3b"] = nc.dram_tensor("dbg_l3b", [97, 33 * 258], f16, kind="ExternalOutput")
        dbg["l4b"] = nc.dram_tensor("dbg_l4b", [128, 32 * 130], f16, kind="ExternalOutput")

    def ap3(t, off, pn, d1s, d1n, d2s, d2n):
        a = t[:]
        return AP(a.tensor, a.offset + off, [[a.ap[0][0], pn], [d1s, d1n], [d2s, d2n]])

    def ap_at(t, p0, pn, off, dims):
        a = t[p0:p0 + pn, :]
        return AP(a.tensor, a.offset + off, [[a.ap[0][0], pn]] + dims)

    with TileContext(nc) as tc:
        with tc.tile_pool(name="const", bufs=1) as cp, \
             tc.tile_pool(name="bufs", bufs=1) as bp, \
             tc.tile_pool(name="work", bufs=4) as wp, \
             tc.tile_pool(name="psum", bufs=1, space="PSUM") as pp:

            wt8 = cp.tile([73, 7 * 128], f8, tag="wt8")
            wt2 = cp.tile([81, 4 * 384], f16, tag="wt2")
            wt3 = cp.tile([97, 3 * 384], f16, tag="wt3")
            wt4 = cp.tile([128, 3 * 384], f16, tag="wt4")
            bias4 = cp.tile([128, 1], f32, tag="bias4")
            nc.sync.dma_start(out=wt8[:], in_=wt8_d[:])

            B8 = bp.tile([73, 34 * 1026], f8, tag="B8", name="B8")
            L2B = bp.tile([81, 34 * 514], f16, tag="L2B", name="L2B")
            L3B = bp.tile([97, 33 * 258], f16, tag="L3B", name="L3B")
            L4B = bp.tile([128, 32 * 130], f16, tag="L4B", name="L4B")

            # ones rows (const-1 bias rhs row for L2/L3)
            nc.sync.dma_start(out=L2B[80:81, :], in_=ones_d[:, 0:34 * 514])
            nc.sync.dma_start(out=L3B[96:97, :], in_=ones_d[:, 0:33 * 258])

            # x-pad zero columns + never-written halo of l2 tile 33
            def pad_memsets(buf, pn, ntiles, w_):
                nc.gpsimd.memset(buf[0:pn, 0:1], 0.0)
                nc.gpsimd.memset(ap_at(buf, 0, pn, w_ - 1, [[w_, ntiles - 1], [1, 2]]), 0.0)
                nc.gpsimd.memset(buf[0:pn, ntiles * w_ - 1:ntiles * w_], 0.0)
            pad_memsets(L2B, 80, 34, 514)
            pad_memsets(L3B, 96, 33, 258)
            pad_memsets(L4B, 128, 32, 130)
            nc.gpsimd.memset(L2B[64:80, 33 * 514:34 * 514], 0.0)

            # preload RELU act table while input DMAs are in flight
            warm = cp.tile([1, 2], f16, tag="warm")
            nc.gpsimd.memset(warm[:].bitcast(f32), 0.0)
            nc.scalar.activation(out=warm[:], in_=warm[:], func=RELU, scale=1.0)

            # input chunks: first small for fast start, weights interleaved early
            chunk_bounds = [0, 2, 6, 12, 18, 24, 29, 34]

            def b8_chunk(ci_):
                c0, c1 = chunk_bounds[ci_], chunk_bounds[ci_ + 1]
                nc.sync.dma_start(out=B8[:, c0 * 1026:c1 * 1026],
                                  in_=b8_d[:, c0 * 1026:c1 * 1026])
            b8_chunk(0)
            b8_chunk(1)
            nc.sync.dma_start(out=wt2[:], in_=wt2_d[:])
            b8_chunk(2)
            for t_, d_ in ((wt3, wt3_d), (wt4, wt4_d), (bias4, bias4_d)):
                nc.sync.dma_start(out=t_[:], in_=d_[:])
            for ci_ in range(3, 7):
                b8_chunk(ci_)

            wb8 = wt8[:]

            def lw(i, j):
                return AP(wb8.tensor, wb8.offset + i * 128,
                          [[wb8.ap[0][0], 73], [(j - i) * 128, 2], [1, 128]])

            b8a = B8[:]

            def pairs(off):
                return AP(b8a.tensor, b8a.offset + off, [[b8a.ap[0][0], 73], [1, 2], [2, 256]])

            L1_PAIRS_E = [(lw(0, 1), 0), (lw(3, 4), 0), (lw(2, 6), 2), (lw(5, 6), 2)]
            L1_PAIRS_O = [(lw(6, 0), 0), (lw(6, 3), 0), (lw(1, 2), 2), (lw(4, 5), 2)]

            def l1_tile(t):
                # tiles 0,1 borrow L2's psum slot (L2 starts at t>=5): depth 3
                # in the latency-critical warmup phase
                if t < 2:
                    P = pp.tile([128, 1024], f32, tag="ps2", name="ps2", bufs=1)
                else:
                    P = pp.tile([128, 1024], f32, tag="ps1", name="ps1", bufs=2)
                for reg, plist in ((0, L1_PAIRS_E), (256, L1_PAIRS_O)):
                    for hx in range(2):
                        base = t * 1026 + 512 * hx
                        o0 = hx * 512 + reg
                        for k, (lhs, poff) in enumerate(plist):
                            nc.tensor.matmul(out=P[:, o0:o0 + 256], lhsT=lhs,
                                             rhs=pairs(base + poff),
                                             start=(k == 0), stop=(k == 3),
                                             perf_mode=DR)
                tmp = wp.tile([128, 512], f16, tag="tmp1", name="tmp1")
                Q = wp.tile([128, 512], f16, tag="q1", name="q1")
                Pt = P[:]
                evens = AP(Pt.tensor, Pt.offset, [[Pt.ap[0][0], 128], [512, 2], [1, 256]])
                odds = AP(Pt.tensor, Pt.offset + 256, [[Pt.ap[0][0], 128], [512, 2], [1, 256]])
                nc.scalar.activation(out=tmp[:].rearrange("p (b x) -> p b x", b=2),
                                     in_=evens, func=RELU, scale=1.0)
                nc.vector.tensor_tensor(out=Q[:].rearrange("p (b x) -> p b x", b=2),
                                        in0=tmp[:].rearrange("p (b x) -> p b x", b=2),
                                        in1=odds, op=AL.max)
                R = wp.tile([64, 512], f16, tag="r1", name="r1")
                if t < 24:
                    nc.vector.tensor_copy(out=R[:], in_=Q[64:128, :])
                else:
                    nc.gpsimd.tensor_copy(out=R[:], in_=Q[64:128, :])
                pend1.append((t, Q, R))
                if len(pend1) > 1:
                    l1_final()

            def l1_final():
                t, Q, R = pend1.pop(0)
                nc.vector.tensor_tensor(out=L2B[0:64, t * 514 + 1:t * 514 + 513],
                                        in0=Q[0:64, :], in1=R[:], op=AL.max)

            def halo12(d0, n):
                nc.sync.dma_start(
                    out=ap_at(L2B, 64, 16, d0 * 514 + 1, [[514, n], [1, 512]]),
                    in_=ap_at(L2B, 0, 16, (d0 + 1) * 514 + 1, [[514, n], [1, 512]]))

            # L2 variant selection: tile -> variant index in wt2 blob
            def l2_var(u):
                return {0: 1, 32: 2, 33: 3}.get(u, 0)

            def l2_chain(c):
                u = 2 * c
                P = pp.tile([128, 1024], f32, tag="ps2", name="ps2", bufs=1)
                for i in range(2):
                    vb = l2_var(u + i) * 384
                    for dx in range(3):
                        nc.tensor.matmul(
                            out=P[:, i * 512:(i + 1) * 512],
                            lhsT=wt2[:, vb + dx * 128:vb + (dx + 1) * 128],
                            rhs=L2B[0:81, (u + i) * 514 + dx:(u + i) * 514 + dx + 512],
                            start=(dx == 0), stop=(dx == 2))
                Pt = P[:]
                evens = AP(Pt.tensor, Pt.offset, [[Pt.ap[0][0], 128], [512, 2], [2, 256]])
                odds = AP(Pt.tensor, Pt.offset + 1, [[Pt.ap[0][0], 128], [512, 2], [2, 256]])
                tmp = wp.tile([128, 512], f16, tag="tmp2", name="tmp2")
                nc.scalar.activation(out=tmp[:].rearrange("p (b x) -> p b x", b=2),
                                     in_=evens, func=RELU, scale=1.0)
                Q = wp.tile([128, 512], f16, tag="q2", name="q2")
                nc.vector.tensor_tensor(out=Q[:].rearrange("p (b x) -> p b x", b=2),
                                        in0=tmp[:].rearrange("p (b x) -> p b x", b=2),
                                        in1=odds, op=AL.max)
                R = wp.tile([64, 512], f16, tag="r2", name="r2")
                if c < 6 or c >= 14:
                    nc.vector.tensor_copy(out=R[:], in_=Q[64:128, :])
                else:
                    nc.gpsimd.tensor_copy(out=R[:], in_=Q[64:128, :])
                pend2.append((c, Q, R))
                if len(pend2) > 1:
                    l2_final()

            def l2h_direct(dst, Q, R, half):
                # dst tile's halo rows (parts 64:96) directly from this final's
                # first 2 pooled rows (parts 0:32) - no DMA hop
                nc.vector.tensor_tensor(
                    out=L3B[64:96, dst * 258 + 1:dst * 258 + 257],
                    in0=Q[0:32, half * 256:half * 256 + 256],
                    in1=R[0:32, half * 256:half * 256 + 256], op=AL.max)

            def l2_final():
                c, Q, R = pend2.pop(0)
                u = 2 * c
                if c < 16:
                    nc.vector.tensor_tensor(
                        out=ap_at(L3B, 0, 64, u * 258 + 1, [[258, 2], [1, 256]]),
                        in0=Q[0:64, :].rearrange("p (b x) -> p b x", b=2),
                        in1=R[:].rearrange("p (b x) -> p b x", b=2), op=AL.max)
                else:
                    nc.vector.tensor_tensor(out=L3B[0:64, 32 * 258 + 1:32 * 258 + 257],
                                            in0=Q[0:64, 0:256], in1=R[:, 0:256], op=AL.max)
                if c == 14:
                    l2h_direct(28, Q, R, 1)
                elif c == 15:
                    l2h_direct(29, Q, R, 0)
                    l2h_direct(30, Q, R, 1)
                elif c == 16:
                    l2h_direct(31, Q, R, 0)
                    l2h_direct(32, Q, R, 1)

            def halo23(d0, n):
                nc.sync.dma_start(
                    out=ap_at(L3B, 64, 32, d0 * 258 + 1, [[258, n], [1, 256]]),
                    in_=ap_at(L3B, 0, 32, (d0 + 1) * 258 + 1, [[258, n], [1, 256]]))

            def l3_var(v):
                return {0: 1, 32: 2}.get(v, 0)

            def l3_chain(d):
                v = 2 * d
                single = (d == 16)
                n = 1 if single else 2
                if d >= 14:
                    Pf = pp.tile([128, 1024], f32, tag="ps2", name="ps2", bufs=1)
                    P = Pf[0:128, 0:512]
                else:
                    P = pp.tile([128, 512], f32, tag="ps34", name="ps34", bufs=2)
                for i in range(n):
                    vb = l3_var(v + i) * 384
                    for dx in range(3):
                        nc.tensor.matmul(
                            out=P[:, i * 256:(i + 1) * 256],
                            lhsT=wt3[:, vb + dx * 128:vb + (dx + 1) * 128],
                            rhs=L3B[0:97, (v + i) * 258 + dx:(v + i) * 258 + dx + 256],
                            start=(dx == 0), stop=(dx == 2))
                Pt = P[:]
                wq = 128 * n
                evens = AP(Pt.tensor, Pt.offset, [[Pt.ap[0][0], 128], [256, n], [2, 128]])
                odds = AP(Pt.tensor, Pt.offset + 1, [[Pt.ap[0][0], 128], [256, n], [2, 128]])
                tmp = wp.tile([128, 256], f16, tag="tmp3", name="tmp3")
                nc.scalar.activation(out=tmp[:, 0:wq].rearrange("p (b x) -> p b x", b=n),
                                     in_=evens, func=RELU, scale=1.0)
                Q = wp.tile([128, 256], f16, tag="q3", name="q3")
                nc.vector.tensor_tensor(out=Q[:, 0:wq].rearrange("p (b x) -> p b x", b=n),
                                        in0=tmp[:, 0:wq].rearrange("p (b x) -> p b x", b=n),
                                        in1=odds, op=AL.max)
                R = wp.tile([64, 256], f16, tag="r3", name="r3")
                if d >= 12:
                    nc.vector.tensor_copy(out=R[:, 0:wq], in_=Q[64:128, 0:wq])
                else:
                    nc.gpsimd.tensor_copy(out=R[:, 0:wq], in_=Q[64:128, 0:wq])
                pend3.append((d, Q, R))
                if len(pend3) > 1:
                    l3_final()

            def l3h_direct(dst, Q, R, half):
                nc.vector.tensor_tensor(
                    out=L4B[64:128, dst * 130 + 1:dst * 130 + 129],
                    in0=Q[0:64, half * 128:half * 128 + 128],
                    in1=R[0:64, half * 128:half * 128 + 128], op=AL.max)

            def l3_final():
                d, Q, R = pend3.pop(0)
                v = 2 * d
                if d < 16:
                    nc.vector.tensor_tensor(
                        out=ap_at(L4B, 0, 64, v * 130 + 1, [[130, 2], [1, 128]]),
                        in0=Q[0:64, 0:256].rearrange("p (b x) -> p b x", b=2),
                        in1=R[:, 0:256].rearrange("p (b x) -> p b x", b=2), op=AL.max)
                if d == 13:
                    l3h_direct(26, Q, R, 1)
                elif d == 14:
                    l3h_direct(27, Q, R, 0)
                    l3h_direct(28, Q, R, 1)
                elif d == 15:
                    l3h_direct(29, Q, R, 0)
                    l3h_direct(30, Q, R, 1)
                elif d == 16:
                    l3h_direct(31, Q, R, 0)

            def halo34(d0, n):
                nc.sync.dma_start(
                    out=ap_at(L4B, 64, 64, d0 * 130 + 1, [[130, n], [1, 128]]),
                    in_=ap_at(L4B, 0, 64, (d0 + 1) * 130 + 1, [[130, n], [1, 128]]))

            def l4_var(w_):
                return {0: 1, 31: 2}.get(w_, 0)

            def l4_chain(e):
                w_ = 4 * e
                if e >= 6:
                    P4full = pp.tile([128, 1024], f32, tag="ps1", name="ps1", bufs=2)
                    P = P4full[0:128, 0:512]
                else:
                    P4full = pp.tile([128, 512], f32, tag="ps34", name="ps34", bufs=2)
                    P = P4full[:]
                for i in range(4):
                    vb = l4_var(w_ + i) * 384
                    for dx in range(3):
                        nc.tensor.matmul(
                            out=P[:, i * 128:(i + 1) * 128],
                            lhsT=wt4[:, vb + dx * 128:vb + (dx + 1) * 128],
                            rhs=L4B[0:128, (w_ + i) * 130 + dx:(w_ + i) * 130 + dx + 128],
                            start=(dx == 0), stop=(dx == 2))
                stage = wp.tile([128, 512], f32, tag="stage", name="stage")
                nc.scalar.activation(out=stage[:], in_=P[:], func=RELU,
                                     bias=bias4[:], scale=1.0)
                sa = stage[:]
                oa = out_d[:]
                pitch = sa.ap[0][0]
                for rp in range(2):
                    in_ap = AP(sa.tensor, sa.offset + rp * pitch,
                               [[2 * pitch, 64], [128, 4], [1, 128]])
                    out_ap = AP(oa.tensor, 1024 * e + 128 * rp,
                                [[8192, 64], [256, 4], [1, 128]])
                    nc.sync.dma_start(out=out_ap, in_=in_ap)

            # ---- emission schedule ----
            # Rate-limited round-robin: at most one unit per layer per step so
            # the in-order PE queue always has other work between same-layer
            # chains (psum bufs=1), with slack lags so halo-DMA latency is
            # hidden.
            state = dict(l2c=0, h23=0, l3c=0, h34=0, l4c=0)
            stage_ref = [None]
            halo12_cov = [0]
            pend1, pend2, pend3 = [], [], []

            def fin2():
                return state['l2c'] - len(pend2)

            def fin3():
                return state['l3c'] - len(pend3)

            def pump(max_per_layer=1):
                done = 0
                # halo23 DMA chunks m=1..14 (dst [2m-2, 2m)); dst 28-32 are
                # direct TT writes in l2_final
                m = state['h23'] + 1
                if m <= 14 and fin2() >= m + 1:
                    halo23(2 * (m - 1), 2); state['h23'] += 1
                # halo34 DMA chunks n=1..13; dst 26-31 direct in l3_final
                nn = state['h34'] + 1
                if nn <= 13 and fin3() >= nn + 1:
                    halo34(2 * (nn - 1), 2); state['h34'] += 1
                for _ in range(max_per_layer):
                    c = state['l2c']
                    if c < 16 and halo12_cov[0] >= min(2 * c + 2, 33):
                        l2_chain(c); state['l2c'] += 1; done += 1
                    elif c == 16 and halo12_cov[0] >= 33:
                        l2_chain(16); state['l2c'] += 1; done += 1
                if pend2 and state['l2c'] == 17:
                    l2_final()
                for _ in range(max_per_layer):
                    d = state['l3c']
                    if d < 14 and 2 * state['h23'] >= min(2 * d + 6, 28):
                        l3_chain(d); state['l3c'] += 1; done += 1
                    elif d == 14 and fin2() >= 16:
                        l3_chain(14); state['l3c'] += 1; done += 1
                    elif d in (15, 16) and fin2() >= 17:
                        l3_chain(d); state['l3c'] += 1; done += 1
                if pend3 and state['l3c'] == 17:
                    l3_final()
                for _ in range(max_per_layer):
                    e = state['l4c']
                    if e < 6 and 2 * state['h34'] >= min(4 * e + 6, 26):
                        l4_chain(e); state['l4c'] += 1; done += 1
                    elif e == 6 and fin3() >= 15 and 2 * state['h34'] >= 26:
                        l4_chain(6); state['l4c'] += 1; done += 1
                    elif e == 7 and fin3() >= 17:
                        l4_chain(7); state['l4c'] += 1; done += 1
                return done

            for t in range(34):
                l1_tile(t)
                # halo12 G=2 at odd t: dst [t-3, t-1), src tiles t-2..t-1
                # (final(t-1) emitted during l1_tile(t))
                if t >= 3 and t % 2 == 1:
                    halo12(t - 3, 2)
                    halo12_cov[0] = t - 1
                pump()
            while pend1:
                l1_final()
            halo12(32, 1)
            halo12_cov[0] = 33
            for _ in range(64):
                if state['l4c'] == 8:
                    break
                pump()
            assert state['l4c'] == 8, state
            assert not pend1 and not pend2 and not pend3

            if debug:
                nc.sync.dma_start(out=dbg["l2b"][:], in_=L2B[:])
                nc.sync.dma_start(out=dbg["l3b"][:], in_=L3B[:])
                nc.sync.dma_start(out=dbg["l4b"][:], in_=L4B[:])

    nc.finalize()
    return nc


def _prep_weights(inputs):
    params = [
        _fold_weights(inputs['w1'], inputs['b1'], inputs['g1'], inputs['be1'], inputs['m1'], inputs['v1']),
        _fold_weights(inputs['w2'], inputs['b2'], inputs['g2'], inputs['be2'], inputs['m2'], inputs['v2']),
        _fold_weights(inputs['w3'], inputs['b3'], inputs['g3'], inputs['be3'], inputs['m3'], inputs['v3']),
        _fold_weights(inputs['w4'], inputs['b4'], inputs['g4'], inputs['be4'], inputs['m4'], inputs['v4']),
    ]
    wt8 = _build_l1_wt(*params[0])
    # variants per h: [main, v_t0, v_t32, v_t33] etc (zsets depend on h)
    wt2 = {}
    wt3 = {}
    wt4 = {}
    for h in (0, 1):
        if h == 0:
            z2 = [[], [0, 1, 2, 3, 4, 5, 6], [], []]
            z3 = [[], [0, 1, 2], []]
            z4 = [[], [0], []]
        else:
            z2 = [[], [], [7, 8, 9], 'all']
            z3 = [[], [], [3, 4, 5]]
            z4 = [[], [], [3]]
        wt2[h] = _build_wt16(params[1][0], params[1][1], 8, 16, 8, 10, False, True, z2)
        wt3[h] = _build_wt16(params[2][0], params[2][1], 16, 32, 4, 6, False, True, z3)
        wt4[h] = _build_wt16(params[3][0], params[3][1], 32, 64, 2, 4, True, False, z4)
    bias4 = np.zeros((128, 1), np.float32)
    bf4 = params[3][1]
    for o in range(64):
        for y in range(2):
            bias4[o * 2 + y, 0] = bf4[o]
    ones = np.ones((1, 34 * 514), np.float16)
    return wt8, wt2, wt3, wt4, bias4, ones


def kernel(points, batch_size,
           w1, b1, g1, be1, m1, v1,
           w2, b2, g2, be2, m2, v2,
           w3, b3, g3, be3, m3, v3,
           w4, b4, g4, be4, m4, v4, **_kw):
    from concourse.bass_utils import run_bass_kernel_spmd

    grids = _bin_points(points)
    inputs = dict(w1=w1, b1=b1, g1=g1, be1=be1, m1=m1, v1=v1,
                  w2=w2, b2=b2, g2=g2, be2=be2, m2=m2, v2=v2,
                  w3=w3, b3=b3, g3=g3, be3=be3, m3=m3, v3=v3,
                  w4=w4, b4=b4, g4=g4, be4=be4, m4=m4, v4=v4)
    wt8, wt2, wt3, wt4, bias4, ones = _prep_weights(inputs)

    core_ids = list(range(8))
    in_maps = []
    for core in core_ids:
        b, h = core // 2, core % 2
        im = {
            "b8": _build_b8(grids[b], h),
            "wt8": wt8,
            "wt2": wt2[h],
            "wt3": wt3[h],
            "wt4": wt4[h],
            "bias4": bias4,
            "ones": ones,
        }
        in_maps.append(im)

    if "nc" not in _CACHE:
        _CACHE["nc"] = _build_module()
    nc = _CACHE["nc"]

    r = run_bass_kernel_spmd(nc, in_maps, core_ids=core_ids)

    out_full = np.zeros((B, 64, 128, 128), np.float32)
    for i, core in enumerate(core_ids):
        b, h = core // 2, core % 2
        out_full[b, :, 64 * h:64 * h + 64, :] = r.results[i]["out"]
    return out_full


# revision 6
# speedup vs baseline: 1.0394x; 1.0026x over previous
"""BEV histogram + 4x(conv3x3+BN+ReLU) + 3x maxpool on 8 trn2 cores, v2.

Sharding: core = 2*b + h computes output rows [64h, 64h+64) of batch b.
Device pipeline per core (all per-layer activations in one SBUF buffer each):

- L1 in fp8 (e4m3): BEV built+quantized on host as [73, 34*1026] (72 rows =
  18 e-rows x 4 ch + const-1 bias row). Conv via x-pair DoubleRow matmuls:
  even/odd output columns computed separately, 2 fp8 weight blocks (hi + lo
  residual) -> 8 DR matmuls per tile = 2N cycles (vs 3N f16).
- L2-4 f16, 3 matmuls/tile, bias as const-1 K row (L4: bias in ACT epilogue).
- Drain per psum chain: ACT relu-copy-even -> DVE TT max(tmp, psum-odd) ->
  fold-copy Q[64:128] (Pool engine, or DVE in latency-critical phases) ->
  DVE final TT max -> next-layer buffer (f16). relu commutes with max
  (max(relu(a), b) == relu(max(a, b)) since relu(a) >= 0); bias is already
  in psum. Finals are emitted one chain late to hide the fold latency from
  the in-order DVE queue.
- Edges (SAME pad at y-borders): per-core *weight data* variants with the
  out-of-range K-rows zeroed - zero device ops.
- Halos: batched SBUF->SBUF DMAs (chunks of 4 tiles) on SP/HWDGE.
"""
import sys
sys.path.insert(0, '/opt/trn_rl_repo')
import numpy as np
import ml_dtypes

PR = [0.0, -39.68, -3.0, 69.12, 39.68, 1.0]
W = 1024
H = 1024
B = 4
BN_EPS = 1e-5
F8 = ml_dtypes.float8_e4m3

_CACHE = {}


def _bin_points(points):
    pts = np.asarray(points, dtype=np.float32)
    xs = np.float32(W / (PR[3] - PR[0]))
    ys = np.float32(H / (PR[4] - PR[1]))
    half = np.float32((PR[4] - PR[1]) / 2)
    xp = (pts[:, 1] * xs).astype(np.int32)
    yp = ((pts[:, 2] + half) * ys).astype(np.int32)
    b = pts[:, 0].astype(np.int32)
    mask = (xp >= 0) & (xp < W) & (yp >= 0) & (yp < H)
    lin = (b * H + yp) * W + xp
    z = pts[:, 3]
    inten = pts[:, 4]
    n = B * H * W
    lv = lin[mask]
    cnt = np.bincount(lv, minlength=n).astype(np.float32)
    zmin = np.full(n, 10.0, np.float32)
    np.minimum.at(zmin, lv, z[mask])
    zmax = np.full(n, -10.0, np.float32)
    np.maximum.at(zmax, lv, z[mask])
    iv = np.zeros(n, np.float32)
    np.maximum.at(iv, lv, inten[mask])
    bev0 = np.where(cnt == 0, np.float32(1.0), cnt) / np.float32(50.0)
    grids = np.stack([bev0, zmin, zmax, iv], axis=0).reshape(4, B, H, W)
    return np.transpose(grids, (1, 0, 2, 3))


def _fold_weights(w, b, g, be, m, v):
    scale = np.asarray(g, np.float32) / np.sqrt(np.asarray(v, np.float32) + np.float32(BN_EPS))
    wf = np.asarray(w, np.float32) * scale[:, None, None, None]
    bf = (np.asarray(b, np.float32) - np.asarray(m, np.float32)) * scale + np.asarray(be, np.float32)
    return wf.astype(np.float32), bf.astype(np.float32)


def _q8(x):
    return np.asarray(x, np.float32).astype(F8).astype(np.float32)


def _build_l1_wt(wf, bf):
    """-> [73, 7*128] e4m3 blob: blocks [w0h,w1h,w2h,w0l,w1l,w2l,Z].
    m = (y%2)*64 + (y//2)*8 + o; bias (hi/lo) on const row of blocks 0/3."""
    whi = _q8(wf)
    wlo = _q8(wf - whi)
    bhi = _q8(bf)
    blo = _q8(bf - bhi)
    blob = np.zeros((73, 7, 128), np.float32)
    for hl, wq in ((0, whi), (1, wlo)):
        for dx in range(3):
            blk = hl * 3 + dx
            for y in range(16):
                m0 = (y % 2) * 64 + (y // 2) * 8
                for dy in range(3):
                    e = y + dy
                    # rows e*4+c ; cols m0+o
                    blob[e * 4:(e + 1) * 4, blk, m0:m0 + 8] = wq[:, :, dy, dx].T
    for y in range(16):
        m0 = (y % 2) * 64 + (y // 2) * 8
        blob[72, 0, m0:m0 + 8] = bhi
        blob[72, 3, m0:m0 + 8] = blo
    return blob.reshape(73, 7 * 128).astype(F8)


def _m_index(y, o, co, co_major):
    if co_major:
        return o * 2 + y
    return (y % 2) * 64 + (y // 2) * co + o


def _build_wt16(wf, bf, ci, co, yoff, eta, co_major, bias_row, variants):
    """-> [K, nvar*384] f16. variants: list of zsets (e-row lists, or 'all')."""
    K = eta * ci + (1 if bias_row else 0)
    main = np.zeros((K, 3, 128), np.float32)
    for dx in range(3):
        for y in range(yoff):
            for dy in range(3):
                e = y + dy
                for o in range(co):
                    m = _m_index(y, o, co, co_major)
                    main[e * ci:(e + 1) * ci, dx, m] = wf[o, :, dy, dx]
    if bias_row:
        for y in range(yoff):
            for o in range(co):
                main[K - 1, 0, _m_index(y, o, co, co_major)] = bf[o]
    blobs = []
    for zset in variants:
        v = main.copy()
        if zset == 'all':
            v[:] = 0.0
        else:
            for e in zset:
                v[e * ci:(e + 1) * ci] = 0.0
        blobs.append(v)
    out = np.concatenate(blobs, axis=1)  # [K, nvar*3, 128]
    return out.reshape(K, -1).astype(np.float16)


def _build_b8(grid_b, h):
    """grid_b [4, 1024, 1024] f32 -> [73, 34*1026] e4m3 (incl ones row)."""
    from numpy.lib.stride_tricks import sliding_window_view
    g0 = 512 * h - 15
    q = np.asarray(grid_b, np.float32).astype(F8).astype(np.float32)
    padded = np.zeros((4, 546, 1026), np.float32)
    lo = max(0, g0)
    hi = min(1024, g0 + 546)
    padded[:, lo - g0:hi - g0, 1:1025] = q[:, lo:hi, :]
    wins = sliding_window_view(padded, 18, axis=1)    # [4, 529, 1026, 18]
    wins = wins[:, 0:16 * 34:16]                      # [4, 34, 1026, 18]
    tiles = np.transpose(wins, (1, 3, 0, 2))          # [34, 18, 4, 1026]
    tiles = np.ascontiguousarray(tiles).reshape(34, 72, 1026)
    ones = np.ones((34, 1, 1026), np.float32)
    full = np.concatenate([tiles, ones], axis=1)      # [34, 73, 1026]
    return np.ascontiguousarray(np.transpose(full, (1, 0, 2))).reshape(73, 34 * 1026).astype(F8)


def _build_module(debug=False):
    import concourse.mybir as mybir
    from concourse.tile import TileContext
    from concourse import bacc
    from concourse.ap import AP

    f32 = mybir.dt.float32
    f16 = mybir.dt.float16
    f8 = mybir.dt.float8e4
    AL = mybir.AluOpType
    RELU = mybir.ActivationFunctionType.Relu
    DR = mybir.MatmulPerfMode.DoubleRow

    nc = bacc.Bacc()
    b8_d = nc.dram_tensor("b8", [73, 34 * 1026], f8, kind="ExternalInput")
    wt8_d = nc.dram_tensor("wt8", [73, 7 * 128], f8, kind="ExternalInput")
    wt2_d = nc.dram_tensor("wt2", [81, 4 * 384], f16, kind="ExternalInput")
    wt3_d = nc.dram_tensor("wt3", [97, 3 * 384], f16, kind="ExternalInput")
    wt4_d = nc.dram_tensor("wt4", [128, 3 * 384], f16, kind="ExternalInput")
    bias4_d = nc.dram_tensor("bias4", [128, 1], f32, kind="ExternalInput")
    ones_d = nc.dram_tensor("ones", [1, 34 * 514], f16, kind="ExternalInput")
    out_d = nc.dram_tensor("out", [64, 64, 128], f32, kind="ExternalOutput")
    dbg = {}
    if debug:
        dbg["l2b"] = nc.dram_tensor("dbg_l2b", [81, 34 * 514], f16, kind="ExternalOutput")
        dbg["l3b"] = nc.dram_tensor("dbg_l3b", [97, 33 * 258], f16, kind="ExternalOutput")
        dbg["l4b"] = nc.dram_tensor("dbg_l4b", [128, 32 * 130], f16, kind="ExternalOutput")

    def ap3(t, off, pn, d1s, d1n, d2s, d2n):
        a = t[:]
        return AP(a.tensor, a.offset + off, [[a.ap[0][0], pn], [d1s, d1n], [d2s, d2n]])

    def ap_at(t, p0, pn, off, dims):
        a = t[p0:p0 + pn, :]
        return AP(a.tensor, a.offset + off, [[a.ap[0][0], pn]] + dims)

    with TileContext(nc) as tc:
        with tc.tile_pool(name="const", bufs=1) as cp, \
             tc.tile_pool(name="bufs", bufs=1) as bp, \
             tc.tile_pool(name="work", bufs=4) as wp, \
             tc.tile_pool(name="psum", bufs=1, space="PSUM") as pp:

            wt8 = cp.tile([73, 7 * 128], f8, tag="wt8")
            wt2 = cp.tile([81, 4 * 384], f16, tag="wt2")
            wt3 = cp.tile([97, 3 * 384], f16, tag="wt3")
            wt4 = cp.tile([128, 3 * 384], f16, tag="wt4")
            bias4 = cp.tile([128, 1], f32, tag="bias4")
            nc.sync.dma_start(out=wt8[:], in_=wt8_d[:])

            B8 = bp.tile([73, 34 * 1026], f8, tag="B8", name="B8")
            L2B = bp.tile([81, 34 * 514], f16, tag="L2B", name="L2B")
            L3B = bp.tile([97, 33 * 258], f16, tag="L3B", name="L3B")
            L4B = bp.tile([128, 32 * 130], f16, tag="L4B", name="L4B")

            # ones rows (const-1 bias rhs row for L2/L3)
            nc.sync.dma_start(out=L2B[80:81, :], in_=ones_d[:, 0:34 * 514])
            nc.sync.dma_start(out=L3B[96:97, :], in_=ones_d[:, 0:33 * 258])

            # x-pad zero columns + never-written halo of l2 tile 33
            def pad_memsets(buf, pn, ntiles, w_):
                nc.gpsimd.memset(buf[0:pn, 0:1], 0.0)
                nc.gpsimd.memset(ap_at(buf, 0, pn, w_ - 1, [[w_, ntiles - 1], [1, 2]]), 0.0)
                nc.gpsimd.memset(buf[0:pn, ntiles * w_ - 1:ntiles * w_], 0.0)
            pad_memsets(L2B, 80, 34, 514)
            pad_memsets(L3B, 96, 33, 258)
            pad_memsets(L4B, 128, 32, 130)
            nc.gpsimd.memset(L2B[64:80, 33 * 514:34 * 514], 0.0)

            # preload RELU act table while input DMAs are in flight
            warm = cp.tile([1, 2], f16, tag="warm")
            nc.gpsimd.memset(warm[:].bitcast(f32), 0.0)
            nc.scalar.activation(out=warm[:], in_=warm[:], func=RELU, scale=1.0)

            # input chunks: first small for fast start, weights interleaved early
            chunk_bounds = [0, 2, 6, 12, 18, 24, 29, 34]

            def b8_chunk(ci_):
                c0, c1 = chunk_bounds[ci_], chunk_bounds[ci_ + 1]
                nc.sync.dma_start(out=B8[:, c0 * 1026:c1 * 1026],
                                  in_=b8_d[:, c0 * 1026:c1 * 1026])
            b8_chunk(0)
            b8_chunk(1)
            nc.sync.dma_start(out=wt2[:], in_=wt2_d[:])
            b8_chunk(2)
            for t_, d_ in ((wt3, wt3_d), (wt4, wt4_d), (bias4, bias4_d)):
                nc.sync.dma_start(out=t_[:], in_=d_[:])
            for ci_ in range(3, 7):
                b8_chunk(ci_)

            wb8 = wt8[:]

            def lw(i, j):
                return AP(wb8.tensor, wb8.offset + i * 128,
                          [[wb8.ap[0][0], 73], [(j - i) * 128, 2], [1, 128]])

            b8a = B8[:]

            def pairs(off):
                return AP(b8a.tensor, b8a.offset + off, [[b8a.ap[0][0], 73], [1, 2], [2, 256]])

            L1_PAIRS_E = [(lw(0, 1), 0), (lw(3, 4), 0), (lw(2, 6), 2), (lw(5, 6), 2)]
            L1_PAIRS_O = [(lw(6, 0), 0), (lw(6, 3), 0), (lw(1, 2), 2), (lw(4, 5), 2)]

            def l1_tile(t):
                # tiles 0,1 borrow L2's psum slot (L2 starts at t>=5): depth 3
                # in the latency-critical warmup phase
                if t < 2:
                    P = pp.tile([128, 1024], f32, tag="ps2", name="ps2", bufs=1)
                else:
                    P = pp.tile([128, 1024], f32, tag="ps1", name="ps1", bufs=2)
                for reg, plist in ((0, L1_PAIRS_E), (256, L1_PAIRS_O)):
                    for hx in range(2):
                        base = t * 1026 + 512 * hx
                        o0 = hx * 512 + reg
                        for k, (lhs, poff) in enumerate(plist):
                            nc.tensor.matmul(out=P[:, o0:o0 + 256], lhsT=lhs,
                                             rhs=pairs(base + poff),
                                             start=(k == 0), stop=(k == 3),
                                             perf_mode=DR)
                tmp = wp.tile([128, 512], f16, tag="tmp1", name="tmp1")
                Q = wp.tile([128, 512], f16, tag="q1", name="q1")
                Pt = P[:]
                evens = AP(Pt.tensor, Pt.offset, [[Pt.ap[0][0], 128], [512, 2], [1, 256]])
                odds = AP(Pt.tensor, Pt.offset + 256, [[Pt.ap[0][0], 128], [512, 2], [1, 256]])
                nc.scalar.activation(out=tmp[:].rearrange("p (b x) -> p b x", b=2),
                                     in_=evens, func=RELU, scale=1.0)
                nc.vector.tensor_tensor(out=Q[:].rearrange("p (b x) -> p b x", b=2),
                                        in0=tmp[:].rearrange("p (b x) -> p b x", b=2),
                                        in1=odds, op=AL.max)
                R = wp.tile([64, 512], f16, tag="r1", name="r1")
                if t < 24:
                    nc.vector.tensor_copy(out=R[:], in_=Q[64:128, :])
                else:
                    nc.gpsimd.tensor_copy(out=R[:], in_=Q[64:128, :])
                pend1.append((t, Q, R))
                if len(pend1) > 1:
                    l1_final()

            def l1_final():
                t, Q, R = pend1.pop(0)
                nc.vector.tensor_tensor(out=L2B[0:64, t * 514 + 1:t * 514 + 513],
                                        in0=Q[0:64, :], in1=R[:], op=AL.max)
                d = t - 1
                if 0 <= d <= 7 or 28 <= d <= 32:
                    # direct halo write (parts 64:80 of dst tile d from this
                    # tile's first 2 pooled rows) - skips the DMA hop
                    nc.vector.tensor_tensor(
                        out=L2B[64:80, d * 514 + 1:d * 514 + 513],
                        in0=Q[0:16, :], in1=R[0:16, :], op=AL.max)
                    halo12_cov[0] = max(halo12_cov[0], d + 1)

            def halo12(d0, n):
                nc.sync.dma_start(
                    out=ap_at(L2B, 64, 16, d0 * 514 + 1, [[514, n], [1, 512]]),
                    in_=ap_at(L2B, 0, 16, (d0 + 1) * 514 + 1, [[514, n], [1, 512]]))

            # L2 variant selection: tile -> variant index in wt2 blob
            def l2_var(u):
                return {0: 1, 32: 2, 33: 3}.get(u, 0)

            def l2_chain(c):
                u = 2 * c
                P = pp.tile([128, 1024], f32, tag="ps2", name="ps2", bufs=1)
                for i in range(2):
                    vb = l2_var(u + i) * 384
                    for dx in range(3):
                        nc.tensor.matmul(
                            out=P[:, i * 512:(i + 1) * 512],
                            lhsT=wt2[:, vb + dx * 128:vb + (dx + 1) * 128],
                            rhs=L2B[0:81, (u + i) * 514 + dx:(u + i) * 514 + dx + 512],
                            start=(dx == 0), stop=(dx == 2))
                Pt = P[:]
                evens = AP(Pt.tensor, Pt.offset, [[Pt.ap[0][0], 128], [512, 2], [2, 256]])
                odds = AP(Pt.tensor, Pt.offset + 1, [[Pt.ap[0][0], 128], [512, 2], [2, 256]])
                tmp = wp.tile([128, 512], f16, tag="tmp2", name="tmp2")
                nc.scalar.activation(out=tmp[:].rearrange("p (b x) -> p b x", b=2),
                                     in_=evens, func=RELU, scale=1.0)
                Q = wp.tile([128, 512], f16, tag="q2", name="q2")
                nc.vector.tensor_tensor(out=Q[:].rearrange("p (b x) -> p b x", b=2),
                                        in0=tmp[:].rearrange("p (b x) -> p b x", b=2),
                                        in1=odds, op=AL.max)
                R = wp.tile([64, 512], f16, tag="r2", name="r2")
                if c < 6 or c >= 14:
                    nc.vector.tensor_copy(out=R[:], in_=Q[64:128, :])
                else:
                    nc.gpsimd.tensor_copy(out=R[:], in_=Q[64:128, :])
                pend2.append((c, Q, R))
                if len(pend2) > 1:
                    l2_final()

            def l2h_direct(dst, Q, R, half):
                # dst tile's halo rows (parts 64:96) directly from this final's
                # first 2 pooled rows (parts 0:32) - no DMA hop
                nc.vector.tensor_tensor(
                    out=L3B[64:96, dst * 258 + 1:dst * 258 + 257],
                    in0=Q[0:32, half * 256:half * 256 + 256],
                    in1=R[0:32, half * 256:half * 256 + 256], op=AL.max)

            def l2_final():
                c, Q, R = pend2.pop(0)
                u = 2 * c
                if c < 16:
                    nc.vector.tensor_tensor(
                        out=ap_at(L3B, 0, 64, u * 258 + 1, [[258, 2], [1, 256]]),
                        in0=Q[0:64, :].rearrange("p (b x) -> p b x", b=2),
                        in1=R[:].rearrange("p (b x) -> p b x", b=2), op=AL.max)
                else:
                    nc.vector.tensor_tensor(out=L3B[0:64, 32 * 258 + 1:32 * 258 + 257],
                                            in0=Q[0:64, 0:256], in1=R[:, 0:256], op=AL.max)
                if c == 14:
                    l2h_direct(28, Q, R, 1)
                elif c == 15:
                    l2h_direct(29, Q, R, 0)
                    l2h_direct(30, Q, R, 1)
                elif c == 16:
                    l2h_direct(31, Q, R, 0)
                    l2h_direct(32, Q, R, 1)

            def halo23(d0, n):
                nc.sync.dma_start(
                    out=ap_at(L3B, 64, 32, d0 * 258 + 1, [[258, n], [1, 256]]),
                    in_=ap_at(L3B, 0, 32, (d0 + 1) * 258 + 1, [[258, n], [1, 256]]))

            def l3_var(v):
                return {0: 1, 32: 2}.get(v, 0)

            def l3_chain(d):
                v = 2 * d
                single = (d == 16)
                n = 1 if single else 2
                if d >= 14:
                    Pf = pp.tile([128, 1024], f32, tag="ps2", name="ps2", bufs=1)
                    P = Pf[0:128, 0:512]
                else:
                    P = pp.tile([128, 512], f32, tag="ps34", name="ps34", bufs=2)
                for i in range(n):
                    vb = l3_var(v + i) * 384
                    for dx in range(3):
                        nc.tensor.matmul(
                            out=P[:, i * 256:(i + 1) * 256],
                            lhsT=wt3[:, vb + dx * 128:vb + (dx + 1) * 128],
                            rhs=L3B[0:97, (v + i) * 258 + dx:(v + i) * 258 + dx + 256],
                            start=(dx == 0), stop=(dx == 2))
                Pt = P[:]
                wq = 128 * n
                evens = AP(Pt.tensor, Pt.offset, [[Pt.ap[0][0], 128], [256, n], [2, 128]])
                odds = AP(Pt.tensor, Pt.offset + 1, [[Pt.ap[0][0], 128], [256, n], [2, 128]])
                tmp = wp.tile([128, 256], f16, tag="tmp3", name="tmp3")
                nc.scalar.activation(out=tmp[:, 0:wq].rearrange("p (b x) -> p b x", b=n),
                                     in_=evens, func=RELU, scale=1.0)
                Q = wp.tile([128, 256], f16, tag="q3", name="q3")
                nc.vector.tensor_tensor(out=Q[:, 0:wq].rearrange("p (b x) -> p b x", b=n),
                                        in0=tmp[:, 0:wq].rearrange("p (b x) -> p b x", b=n),
                                        in1=odds, op=AL.max)
                R = wp.tile([64, 256], f16, tag="r3", name="r3")
                if d >= 12:
                    nc.vector.tensor_copy(out=R[:, 0:wq], in_=Q[64:128, 0:wq])
                else:
                    nc.gpsimd.tensor_copy(out=R[:, 0:wq], in_=Q[64:128, 0:wq])
                pend3.append((d, Q, R))
                if len(pend3) > 1:
                    l3_final()

            def l3h_direct(dst, Q, R, half):
                nc.vector.tensor_tensor(
                    out=L4B[64:128, dst * 130 + 1:dst * 130 + 129],
                    in0=Q[0:64, half * 128:half * 128 + 128],
                    in1=R[0:64, half * 128:half * 128 + 128], op=AL.max)

            def l3_final():
                d, Q, R = pend3.pop(0)
                v = 2 * d
                if d < 16:
                    nc.vector.tensor_tensor(
                        out=ap_at(L4B, 0, 64, v * 130 + 1, [[130, 2], [1, 128]]),
                        in0=Q[0:64, 0:256].rearrange("p (b x) -> p b x", b=2),
                        in1=R[:, 0:256].rearrange("p (b x) -> p b x", b=2), op=AL.max)
                if d == 13:
                    l3h_direct(26, Q, R, 1)
                elif d == 14:
                    l3h_direct(27, Q, R, 0)
                    l3h_direct(28, Q, R, 1)
                elif d == 15:
                    l3h_direct(29, Q, R, 0)
                    l3h_direct(30, Q, R, 1)
                elif d == 16:
                    l3h_direct(31, Q, R, 0)

            def halo34(d0, n):
                nc.sync.dma_start(
                    out=ap_at(L4B, 64, 64, d0 * 130 + 1, [[130, n], [1, 128]]),
                    in_=ap_at(L4B, 0, 64, (d0 + 1) * 130 + 1, [[130, n], [1, 128]]))

            def l4_var(w_):
                return {0: 1, 31: 2}.get(w_, 0)

            def l4_chain(e):
                w_ = 4 * e
                if e >= 6:
                    P4full = pp.tile([128, 1024], f32, tag="ps1", name="ps1", bufs=2)
                    P = P4full[0:128, 0:512]
                else:
                    P4full = pp.tile([128, 512], f32, tag="ps34", name="ps34", bufs=2)
                    P = P4full[:]
                for i in range(4):
                    vb = l4_var(w_ + i) * 384
                    for dx in range(3):
                        nc.tensor.matmul(
                            out=P[:, i * 128:(i + 1) * 128],
                            lhsT=wt4[:, vb + dx * 128:vb + (dx + 1) * 128],
                            rhs=L4B[0:128, (w_ + i) * 130 + dx:(w_ + i) * 130 + dx + 128],
                            start=(dx == 0), stop=(dx == 2))
                stage = wp.tile([128, 512], f32, tag="stage", name="stage")
                nc.scalar.activation(out=stage[:], in_=P[:], func=RELU,
                                     bias=bias4[:], scale=1.0)
                sa = stage[:]
                oa = out_d[:]
                pitch = sa.ap[0][0]
                for rp in range(2):
                    in_ap = AP(sa.tensor, sa.offset + rp * pitch,
                               [[2 * pitch, 64], [128, 4], [1, 128]])
                    out_ap = AP(oa.tensor, 1024 * e + 128 * rp,
                                [[8192, 64], [256, 4], [1, 128]])
                    nc.sync.dma_start(out=out_ap, in_=in_ap)

            # ---- emission schedule ----
            # Rate-limited round-robin: at most one unit per layer per step so
            # the in-order PE queue always has other work between same-layer
            # chains (psum bufs=1), with slack lags so halo-DMA latency is
            # hidden.
            state = dict(l2c=0, h23=0, l3c=0, h34=0, l4c=0)
            stage_ref = [None]
            halo12_cov = [0]
            pend1, pend2, pend3 = [], [], []

            def fin2():
                return state['l2c'] - len(pend2)

            def fin3():
                return state['l3c'] - len(pend3)

            def pump(max_per_layer=1):
                done = 0
                # halo23 DMA chunks m=1..14 (dst [2m-2, 2m)); dst 28-32 are
                # direct TT writes in l2_final
                m = state['h23'] + 1
                if m <= 14 and fin2() >= m + 1:
                    halo23(2 * (m - 1), 2); state['h23'] += 1
                # halo34 DMA chunks n=1..13; dst 26-31 direct in l3_final
                nn = state['h34'] + 1
                if nn <= 13 and fin3() >= nn + 1:
                    halo34(2 * (nn - 1), 2); state['h34'] += 1
                for _ in range(max_per_layer):
                    c = state['l2c']
                    if c < 16 and halo12_cov[0] >= min(2 * c + 2, 33):
                        l2_chain(c); state['l2c'] += 1; done += 1
                    elif c == 16 and halo12_cov[0] >= 33:
                        l2_chain(16); state['l2c'] += 1; done += 1
                if pend2 and state['l2c'] == 17:
                    l2_final()
                for _ in range(max_per_layer):
                    d = state['l3c']
                    if d < 14 and 2 * state['h23'] >= min(2 * d + 6, 28):
                        l3_chain(d); state['l3c'] += 1; done += 1
                    elif d == 14 and fin2() >= 16:
                        l3_chain(14); state['l3c'] += 1; done += 1
                    elif d in (15, 16) and fin2() >= 17:
                        l3_chain(d); state['l3c'] += 1; done += 1
                if pend3 and state['l3c'] == 17:
                    l3_final()
                for _ in range(max_per_layer):
                    e = state['l4c']
                    if e < 6 and 2 * state['h34'] >= min(4 * e + 6, 26):
                        l4_chain(e); state['l4c'] += 1; done += 1
                    elif e == 6 and fin3() >= 15 and 2 * state['h34'] >= 26:
                        l4_chain(6); state['l4c'] += 1; done += 1
                    elif e == 7 and fin3() >= 17:
                        l4_chain(7); state['l4c'] += 1; done += 1
                return done

            for t in range(34):
                l1_tile(t)
                # halo12 G=2 DMA chunks for mid-range dst 8..27 at odd t;
                # dst 0-7 and 28-32 are direct TT writes in l1_final
                if 11 <= t <= 29 and t % 2 == 1:
                    halo12(t - 3, 2)
                    halo12_cov[0] = max(halo12_cov[0], t - 1)
                pump()
            while pend1:
                l1_final()
            halo12_cov[0] = 33
            for _ in range(64):
                if state['l4c'] == 8:
                    break
                pump()
            assert state['l4c'] == 8, state
            assert not pend1 and not pend2 and not pend3

            if debug:
                nc.sync.dma_start(out=dbg["l2b"][:], in_=L2B[:])
                nc.sync.dma_start(out=dbg["l3b"][:], in_=L3B[:])
                nc.sync.dma_start(out=dbg["l4b"][:], in_=L4B[:])

    nc.finalize()
    return nc


def _prep_weights(inputs):
    params = [
        _fold_weights(inputs['w1'], inputs['b1'], inputs['g1'], inputs['be1'], inputs['m1'], inputs['v1']),
        _fold_weights(inputs['w2'], inputs['b2'], inputs['g2'], inputs['be2'], inputs['m2'], inputs['v2']),
        _fold_weights(inputs['w3'], inputs['b3'], inputs['g3'], inputs['be3'], inputs['m3'], inputs['v3']),
        _fold_weights(inputs['w4'], inputs['b4'], inputs['g4'], inputs['be4'], inputs['m4'], inputs['v4']),
    ]
    wt8 = _build_l1_wt(*params[0])
    # variants per h: [main, v_t0, v_t32, v_t33] etc (zsets depend on h)
    wt2 = {}
    wt3 = {}
    wt4 = {}
    for h in (0, 1):
        if h == 0:
            z2 = [[], [0, 1, 2, 3, 4, 5, 6], [], []]
            z3 = [[], [0, 1, 2], []]
            z4 = [[], [0], []]
        else:
            z2 = [[], [], [7, 8, 9], 'all']
            z3 = [[], [], [3, 4, 5]]
            z4 = [[], [], [3]]
        wt2[h] = _build_wt16(params[1][0], params[1][1], 8, 16, 8, 10, False, True, z2)
        wt3[h] = _build_wt16(params[2][0], params[2][1], 16, 32, 4, 6, False, True, z3)
        wt4[h] = _build_wt16(params[3][0], params[3][1], 32, 64, 2, 4, True, False, z4)
    bias4 = np.zeros((128, 1), np.float32)
    bf4 = params[3][1]
    for o in range(64):
        for y in range(2):
            bias4[o * 2 + y, 0] = bf4[o]
    ones = np.ones((1, 34 * 514), np.float16)
    return wt8, wt2, wt3, wt4, bias4, ones


def kernel(points, batch_size,
           w1, b1, g1, be1, m1, v1,
           w2, b2, g2, be2, m2, v2,
           w3, b3, g3, be3, m3, v3,
           w4, b4, g4, be4, m4, v4, **_kw):
    from concourse.bass_utils import run_bass_kernel_spmd

    grids = _bin_points(points)
    inputs = dict(w1=w1, b1=b1, g1=g1, be1=be1, m1=m1, v1=v1,
                  w2=w2, b2=b2, g2=g2, be2=be2, m2=m2, v2=v2,
                  w3=w3, b3=b3, g3=g3, be3=be3, m3=m3, v3=v3,
                  w4=w4, b4=b4, g4=g4, be4=be4, m4=m4, v4=v4)
    wt8, wt2, wt3, wt4, bias4, ones = _prep_weights(inputs)

    core_ids = list(range(8))
    in_maps = []
    for core in core_ids:
        b, h = core // 2, core % 2
        im = {
            "b8": _build_b8(grids[b], h),
            "wt8": wt8,
            "wt2": wt2[h],
            "wt3": wt3[h],
            "wt4": wt4[h],
            "bias4": bias4,
            "ones": ones,
        }
        in_maps.append(im)

    if "nc" not in _CACHE:
        _CACHE["nc"] = _build_module()
    nc = _CACHE["nc"]

    r = run_bass_kernel_spmd(nc, in_maps, core_ids=core_ids)

    out_full = np.zeros((B, 64, 128, 128), np.float32)
    for i, core in enumerate(core_ids):
        b, h = core // 2, core % 2
        out_full[b, :, 64 * h:64 * h + 64, :] = r.results[i]["out"]
    return out_full


# revision 8
# speedup vs baseline: 1.0720x; 1.0314x over previous
"""BEV histogram + 4x(conv3x3+BN+ReLU) + 3x maxpool on 8 trn2 cores, v2.

Sharding: core = 2*b + h computes output rows [64h, 64h+64) of batch b.
Device pipeline per core (all per-layer activations in one SBUF buffer each):

- L1 in fp8 (e4m3): BEV built+quantized on host as [73, 34*1026] (72 rows =
  18 e-rows x 4 ch + const-1 bias row). Conv via x-pair DoubleRow matmuls:
  even/odd output columns computed separately, 2 fp8 weight blocks (hi + lo
  residual) -> 8 DR matmuls per tile = 2N cycles (vs 3N f16).
- L2-4 f16, 3 matmuls/tile, bias as const-1 K row (L4: bias in ACT epilogue).
- Drain per psum chain: ACT relu-copy-even -> DVE TT max(tmp, psum-odd) ->
  fold-copy Q[64:128] (Pool engine, or DVE in latency-critical phases) ->
  DVE final TT max -> next-layer buffer (f16). relu commutes with max
  (max(relu(a), b) == relu(max(a, b)) since relu(a) >= 0); bias is already
  in psum. Finals are emitted one chain late to hide the fold latency from
  the in-order DVE queue.
- Edges (SAME pad at y-borders): per-core *weight data* variants with the
  out-of-range K-rows zeroed - zero device ops.
- Halos: mid-pipeline via batched SBUF->SBUF DMAs on SP/HWDGE; warmup and
  tail halos via direct partition-shifted TT writes (engine outputs may start
  at any partition; only both-SBUF *inputs* must share a base partition) -
  this removes every DMA hop from the pipeline-drain critical path.
"""
import sys
sys.path.insert(0, '/opt/trn_rl_repo')
import numpy as np
import ml_dtypes

PR = [0.0, -39.68, -3.0, 69.12, 39.68, 1.0]
W = 1024
H = 1024
B = 4
BN_EPS = 1e-5
F8 = ml_dtypes.float8_e4m3

_CACHE = {}


def _bin_points(points):
    pts = np.asarray(points, dtype=np.float32)
    xs = np.float32(W / (PR[3] - PR[0]))
    ys = np.float32(H / (PR[4] - PR[1]))
    half = np.float32((PR[4] - PR[1]) / 2)
    xp = (pts[:, 1] * xs).astype(np.int32)
    yp = ((pts[:, 2] + half) * ys).astype(np.int32)
    b = pts[:, 0].astype(np.int32)
    mask = (xp >= 0) & (xp < W) & (yp >= 0) & (yp < H)
    lin = (b * H + yp) * W + xp
    z = pts[:, 3]
    inten = pts[:, 4]
    n = B * H * W
    lv = lin[mask]
    cnt = np.bincount(lv, minlength=n).astype(np.float32)
    zmin = np.full(n, 10.0, np.float32)
    np.minimum.at(zmin, lv, z[mask])
    zmax = np.full(n, -10.0, np.float32)
    np.maximum.at(zmax, lv, z[mask])
    iv = np.zeros(n, np.float32)
    np.maximum.at(iv, lv, inten[mask])
    bev0 = np.where(cnt == 0, np.float32(1.0), cnt) / np.float32(50.0)
    grids = np.stack([bev0, zmin, zmax, iv], axis=0).reshape(4, B, H, W)
    return np.transpose(grids, (1, 0, 2, 3))


def _fold_weights(w, b, g, be, m, v):
    scale = np.asarray(g, np.float32) / np.sqrt(np.asarray(v, np.float32) + np.float32(BN_EPS))
    wf = np.asarray(w, np.float32) * scale[:, None, None, None]
    bf = (np.asarray(b, np.float32) - np.asarray(m, np.float32)) * scale + np.asarray(be, np.float32)
    return wf.astype(np.float32), bf.astype(np.float32)


def _q8(x):
    return np.asarray(x, np.float32).astype(F8).astype(np.float32)


def _build_l1_wt(wf, bf):
    """-> [73, 7*128] e4m3 blob: blocks [w0h,w1h,w2h,w0l,w1l,w2l,Z].
    m = (y%2)*64 + (y//2)*8 + o; bias (hi/lo) on const row of blocks 0/3."""
    whi = _q8(wf)
    wlo = _q8(wf - whi)
    bhi = _q8(bf)
    blo = _q8(bf - bhi)
    blob = np.zeros((73, 7, 128), np.float32)
    for hl, wq in ((0, whi), (1, wlo)):
        for dx in range(3):
            blk = hl * 3 + dx
            for y in range(16):
                m0 = (y % 2) * 64 + (y // 2) * 8
                for dy in range(3):
                    e = y + dy
                    # rows e*4+c ; cols m0+o
                    blob[e * 4:(e + 1) * 4, blk, m0:m0 + 8] = wq[:, :, dy, dx].T
    for y in range(16):
        m0 = (y % 2) * 64 + (y // 2) * 8
        blob[72, 0, m0:m0 + 8] = bhi
        blob[72, 3, m0:m0 + 8] = blo
    return blob.reshape(73, 7 * 128).astype(F8)


def _m_index(y, o, co, co_major):
    if co_major:
        return o * 2 + y
    return (y % 2) * 64 + (y // 2) * co + o


def _build_wt16(wf, bf, ci, co, yoff, eta, co_major, bias_row, variants):
    """-> [K, nvar*384] f16. variants: list of zsets (e-row lists, or 'all')."""
    K = eta * ci + (1 if bias_row else 0)
    main = np.zeros((K, 3, 128), np.float32)
    for dx in range(3):
        for y in range(yoff):
            for dy in range(3):
                e = y + dy
                for o in range(co):
                    m = _m_index(y, o, co, co_major)
                    main[e * ci:(e + 1) * ci, dx, m] = wf[o, :, dy, dx]
    if bias_row:
        for y in range(yoff):
            for o in range(co):
                main[K - 1, 0, _m_index(y, o, co, co_major)] = bf[o]
    blobs = []
    for zset in variants:
        v = main.copy()
        if zset == 'all':
            v[:] = 0.0
        else:
            for e in zset:
                v[e * ci:(e + 1) * ci] = 0.0
        blobs.append(v)
    out = np.concatenate(blobs, axis=1)  # [K, nvar*3, 128]
    return out.reshape(K, -1).astype(np.float16)


def _build_b8(grid_b, h):
    """grid_b [4, 1024, 1024] f32 -> [73, 34*1026] e4m3 (incl ones row)."""
    from numpy.lib.stride_tricks import sliding_window_view
    g0 = 512 * h - 15
    q = np.asarray(grid_b, np.float32).astype(F8).astype(np.float32)
    padded = np.zeros((4, 546, 1026), np.float32)
    lo = max(0, g0)
    hi = min(1024, g0 + 546)
    padded[:, lo - g0:hi - g0, 1:1025] = q[:, lo:hi, :]
    wins = sliding_window_view(padded, 18, axis=1)    # [4, 529, 1026, 18]
    wins = wins[:, 0:16 * 34:16]                      # [4, 34, 1026, 18]
    tiles = np.transpose(wins, (1, 3, 0, 2))          # [34, 18, 4, 1026]
    tiles = np.ascontiguousarray(tiles).reshape(34, 72, 1026)
    ones = np.ones((34, 1, 1026), np.float32)
    full = np.concatenate([tiles, ones], axis=1)      # [34, 73, 1026]
    return np.ascontiguousarray(np.transpose(full, (1, 0, 2))).reshape(73, 34 * 1026).astype(F8)


def _build_module(debug=False):
    import concourse.mybir as mybir
    from concourse.tile import TileContext
    from concourse import bacc
    from concourse.ap import AP

    f32 = mybir.dt.float32
    f16 = mybir.dt.float16
    f8 = mybir.dt.float8e4
    AL = mybir.AluOpType
    RELU = mybir.ActivationFunctionType.Relu
    DR = mybir.MatmulPerfMode.DoubleRow

    nc = bacc.Bacc()
    b8_d = nc.dram_tensor("b8", [73, 34 * 1026], f8, kind="ExternalInput")
    wt8_d = nc.dram_tensor("wt8", [73, 7 * 128], f8, kind="ExternalInput")
    wt2_d = nc.dram_tensor("wt2", [81, 4 * 384], f16, kind="ExternalInput")
    wt3_d = nc.dram_tensor("wt3", [97, 3 * 384], f16, kind="ExternalInput")
    wt4_d = nc.dram_tensor("wt4", [128, 3 * 384], f16, kind="ExternalInput")
    bias4_d = nc.dram_tensor("bias4", [128, 1], f32, kind="ExternalInput")
    ones_d = nc.dram_tensor("ones", [1, 34 * 514], f16, kind="ExternalInput")
    out_d = nc.dram_tensor("out", [64, 64, 128], f32, kind="ExternalOutput")
    dbg = {}
    if debug:
        dbg["l2b"] = nc.dram_tensor("dbg_l2b", [81, 34 * 514], f16, kind="ExternalOutput")
        dbg["l3b"] = nc.dram_tensor("dbg_l3b", [97, 33 * 258], f16, kind="ExternalOutput")
        dbg["l4b"] = nc.dram_tensor("dbg_l4b", [128, 32 * 130], f16, kind="ExternalOutput")

    def ap3(t, off, pn, d1s, d1n, d2s, d2n):
        a = t[:]
        return AP(a.tensor, a.offset + off, [[a.ap[0][0], pn], [d1s, d1n], [d2s, d2n]])

    def ap_at(t, p0, pn, off, dims):
        a = t[p0:p0 + pn, :]
        return AP(a.tensor, a.offset + off, [[a.ap[0][0], pn]] + dims)

    with TileContext(nc) as tc:
        with tc.tile_pool(name="const", bufs=1) as cp, \
             tc.tile_pool(name="bufs", bufs=1) as bp, \
             tc.tile_pool(name="work", bufs=6) as wp, \
             tc.tile_pool(name="psum", bufs=1, space="PSUM") as pp:

            wt8 = cp.tile([73, 7 * 128], f8, tag="wt8")
            wt2 = cp.tile([81, 4 * 384], f16, tag="wt2")
            wt3 = cp.tile([97, 3 * 384], f16, tag="wt3")
            wt4 = cp.tile([128, 3 * 384], f16, tag="wt4")
            bias4 = cp.tile([128, 1], f32, tag="bias4")
            nc.sync.dma_start(out=wt8[:], in_=wt8_d[:])

            B8 = bp.tile([73, 34 * 1026], f8, tag="B8", name="B8")
            L2B = bp.tile([81, 34 * 514], f16, tag="L2B", name="L2B")
            L3B = bp.tile([97, 33 * 258], f16, tag="L3B", name="L3B")
            L4B = bp.tile([128, 32 * 130], f16, tag="L4B", name="L4B")

            # ones rows (const-1 bias rhs row for L2/L3)
            nc.sync.dma_start(out=L2B[80:81, :], in_=ones_d[:, 0:34 * 514])
            nc.sync.dma_start(out=L3B[96:97, :], in_=ones_d[:, 0:33 * 258])

            # x-pad zero columns + never-written halo of l2 tile 33
            def pad_memsets(buf, pn, ntiles, w_):
                nc.gpsimd.memset(buf[0:pn, 0:1], 0.0)
                nc.gpsimd.memset(ap_at(buf, 0, pn, w_ - 1, [[w_, ntiles - 1], [1, 2]]), 0.0)
                nc.gpsimd.memset(buf[0:pn, ntiles * w_ - 1:ntiles * w_], 0.0)
            pad_memsets(L2B, 80, 34, 514)
            pad_memsets(L3B, 96, 33, 258)
            pad_memsets(L4B, 128, 32, 130)
            nc.gpsimd.memset(L2B[64:80, 33 * 514:34 * 514], 0.0)

            # preload RELU act table while input DMAs are in flight
            warm = cp.tile([1, 2], f16, tag="warm")
            nc.gpsimd.memset(warm[:].bitcast(f32), 0.0)
            nc.scalar.activation(out=warm[:], in_=warm[:], func=RELU, scale=1.0)

            # input chunks: first small for fast start, weights interleaved early
            chunk_bounds = [0, 2, 6, 12, 18, 24, 29, 34]

            def b8_chunk(ci_):
                c0, c1 = chunk_bounds[ci_], chunk_bounds[ci_ + 1]
                nc.sync.dma_start(out=B8[:, c0 * 1026:c1 * 1026],
                                  in_=b8_d[:, c0 * 1026:c1 * 1026])
            b8_chunk(0)
            b8_chunk(1)
            nc.sync.dma_start(out=wt2[:], in_=wt2_d[:])
            b8_chunk(2)
            for t_, d_ in ((wt3, wt3_d), (wt4, wt4_d), (bias4, bias4_d)):
                nc.sync.dma_start(out=t_[:], in_=d_[:])
            for ci_ in range(3, 7):
                b8_chunk(ci_)

            wb8 = wt8[:]

            def lw(i, j):
                return AP(wb8.tensor, wb8.offset + i * 128,
                          [[wb8.ap[0][0], 73], [(j - i) * 128, 2], [1, 128]])

            b8a = B8[:]

            def pairs(off):
                return AP(b8a.tensor, b8a.offset + off, [[b8a.ap[0][0], 73], [1, 2], [2, 256]])

            L1_PAIRS_E = [(lw(0, 1), 0), (lw(3, 4), 0), (lw(2, 6), 2), (lw(5, 6), 2)]
            L1_PAIRS_O = [(lw(6, 0), 0), (lw(6, 3), 0), (lw(1, 2), 2), (lw(4, 5), 2)]

            def l1_tile(t):
                # tiles 0,1 borrow L2's psum slot (L2 starts at t>=5): depth 3
                # in the latency-critical warmup phase
                if t < 2:
                    P = pp.tile([128, 1024], f32, tag="ps2", name="ps2", bufs=1)
                else:
                    P = pp.tile([128, 1024], f32, tag="ps1", name="ps1", bufs=2)
                for reg, plist in ((0, L1_PAIRS_E), (256, L1_PAIRS_O)):
                    for hx in range(2):
                        base = t * 1026 + 512 * hx
                        o0 = hx * 512 + reg
                        for k, (lhs, poff) in enumerate(plist):
                            nc.tensor.matmul(out=P[:, o0:o0 + 256], lhsT=lhs,
                                             rhs=pairs(base + poff),
                                             start=(k == 0), stop=(k == 3),
                                             perf_mode=DR)
                tmp = wp.tile([128, 512], f16, tag="tmp1", name="tmp1")
                Q = wp.tile([128, 512], f16, tag="q1", name="q1")
                Pt = P[:]
                evens = AP(Pt.tensor, Pt.offset, [[Pt.ap[0][0], 128], [512, 2], [1, 256]])
                odds = AP(Pt.tensor, Pt.offset + 256, [[Pt.ap[0][0], 128], [512, 2], [1, 256]])
                nc.scalar.activation(out=tmp[:].rearrange("p (b x) -> p b x", b=2),
                                     in_=evens, func=RELU, scale=1.0)
                nc.vector.tensor_tensor(out=Q[:].rearrange("p (b x) -> p b x", b=2),
                                        in0=tmp[:].rearrange("p (b x) -> p b x", b=2),
                                        in1=odds, op=AL.max)
                R = wp.tile([64, 512], f16, tag="r1", name="r1")
                if t < 24:
                    nc.vector.tensor_copy(out=R[:], in_=Q[64:128, :])
                else:
                    nc.gpsimd.tensor_copy(out=R[:], in_=Q[64:128, :])
                pend1.append((t, Q, R))
                if len(pend1) > 1:
                    l1_final()

            def l1_final():
                t, Q, R = pend1.pop(0)
                nc.vector.tensor_tensor(out=L2B[0:64, t * 514 + 1:t * 514 + 513],
                                        in0=Q[0:64, :], in1=R[:], op=AL.max)
                d = t - 1
                if 8 <= d <= 27:
                    # halo = first 2 pooled rows of tile t (just written):
                    # ACT copy instead of a DMA hop
                    nc.scalar.copy(out=L2B[64:80, d * 514 + 1:d * 514 + 513],
                                   in_=L2B[0:16, t * 514 + 1:t * 514 + 513])
                    halo12_cov[0] = max(halo12_cov[0], d + 1)
                if 0 <= d <= 7 or 28 <= d <= 32:
                    # direct halo write (parts 64:80 of dst tile d from this
                    # tile's first 2 pooled rows) - skips the DMA hop
                    nc.vector.tensor_tensor(
                        out=L2B[64:80, d * 514 + 1:d * 514 + 513],
                        in0=Q[0:16, :], in1=R[0:16, :], op=AL.max)
                    halo12_cov[0] = max(halo12_cov[0], d + 1)

            def halo12(d0, n):
                nc.sync.dma_start(
                    out=ap_at(L2B, 64, 16, d0 * 514 + 1, [[514, n], [1, 512]]),
                    in_=ap_at(L2B, 0, 16, (d0 + 1) * 514 + 1, [[514, n], [1, 512]]))

            # L2 variant selection: tile -> variant index in wt2 blob
            def l2_var(u):
                return {0: 1, 32: 2, 33: 3}.get(u, 0)

            def l2_chain(c):
                u = 2 * c
                P = pp.tile([128, 1024], f32, tag="ps2", name="ps2", bufs=1)
                for i in range(2):
                    vb = l2_var(u + i) * 384
                    for dx in range(3):
                        nc.tensor.matmul(
                            out=P[:, i * 512:(i + 1) * 512],
                            lhsT=wt2[:, vb + dx * 128:vb + (dx + 1) * 128],
                            rhs=L2B[0:81, (u + i) * 514 + dx:(u + i) * 514 + dx + 512],
                            start=(dx == 0), stop=(dx == 2))
                Pt = P[:]
                evens = AP(Pt.tensor, Pt.offset, [[Pt.ap[0][0], 128], [512, 2], [2, 256]])
                odds = AP(Pt.tensor, Pt.offset + 1, [[Pt.ap[0][0], 128], [512, 2], [2, 256]])
                tmp = wp.tile([128, 512], f16, tag="tmp2", name="tmp2")
                nc.scalar.activation(out=tmp[:].rearrange("p (b x) -> p b x", b=2),
                                     in_=evens, func=RELU, scale=1.0)
                Q = wp.tile([128, 512], f16, tag="q2", name="q2")
                nc.vector.tensor_tensor(out=Q[:].rearrange("p (b x) -> p b x", b=2),
                                        in0=tmp[:].rearrange("p (b x) -> p b x", b=2),
                                        in1=odds, op=AL.max)
                R = wp.tile([64, 512], f16, tag="r2", name="r2")
                if c < 4 or c >= 12:
                    nc.vector.tensor_copy(out=R[:], in_=Q[64:128, :])
                else:
                    nc.gpsimd.tensor_copy(out=R[:], in_=Q[64:128, :])
                pend2.append((c, Q, R))
                if len(pend2) > 1:
                    l2_final()

            def l2h_direct(dst, Q, R, half):
                # dst tile's halo rows (parts 64:96) directly from this final's
                # first 2 pooled rows (parts 0:32) - no DMA hop
                nc.vector.tensor_tensor(
                    out=L3B[64:96, dst * 258 + 1:dst * 258 + 257],
                    in0=Q[0:32, half * 256:half * 256 + 256],
                    in1=R[0:32, half * 256:half * 256 + 256], op=AL.max)

            def l2_final():
                c, Q, R = pend2.pop(0)
                u = 2 * c
                if c < 16:
                    nc.vector.tensor_tensor(
                        out=ap_at(L3B, 0, 64, u * 258 + 1, [[258, 2], [1, 256]]),
                        in0=Q[0:64, :].rearrange("p (b x) -> p b x", b=2),
                        in1=R[:].rearrange("p (b x) -> p b x", b=2), op=AL.max)
                else:
                    nc.vector.tensor_tensor(out=L3B[0:64, 32 * 258 + 1:32 * 258 + 257],
                                            in0=Q[0:64, 0:256], in1=R[:, 0:256], op=AL.max)
                if c == 14:
                    l2h_direct(28, Q, R, 1)
                elif c == 15:
                    l2h_direct(29, Q, R, 0)
                    l2h_direct(30, Q, R, 1)
                elif c == 16:
                    l2h_direct(31, Q, R, 0)
                    l2h_direct(32, Q, R, 1)

            def halo23(d0, n):
                nc.sync.dma_start(
                    out=ap_at(L3B, 64, 32, d0 * 258 + 1, [[258, n], [1, 256]]),
                    in_=ap_at(L3B, 0, 32, (d0 + 1) * 258 + 1, [[258, n], [1, 256]]))

            def l3_var(v):
                return {0: 1, 32: 2}.get(v, 0)

            def l3_chain(d):
                v = 2 * d
                single = (d == 16)
                n = 1 if single else 2
                if d >= 14:
                    Pf = pp.tile([128, 1024], f32, tag="ps2", name="ps2", bufs=1)
                    P = Pf[0:128, 0:512]
                else:
                    P = pp.tile([128, 512], f32, tag="ps34", name="ps34", bufs=2)
                for i in range(n):
                    vb = l3_var(v + i) * 384
                    for dx in range(3):
                        nc.tensor.matmul(
                            out=P[:, i * 256:(i + 1) * 256],
                            lhsT=wt3[:, vb + dx * 128:vb + (dx + 1) * 128],
                            rhs=L3B[0:97, (v + i) * 258 + dx:(v + i) * 258 + dx + 256],
                            start=(dx == 0), stop=(dx == 2))
                Pt = P[:]
                wq = 128 * n
                evens = AP(Pt.tensor, Pt.offset, [[Pt.ap[0][0], 128], [256, n], [2, 128]])
                odds = AP(Pt.tensor, Pt.offset + 1, [[Pt.ap[0][0], 128], [256, n], [2, 128]])
                tmp = wp.tile([128, 256], f16, tag="tmp3", name="tmp3")
                nc.scalar.activation(out=tmp[:, 0:wq].rearrange("p (b x) -> p b x", b=n),
                                     in_=evens, func=RELU, scale=1.0)
                Q = wp.tile([128, 256], f16, tag="q3", name="q3")
                nc.vector.tensor_tensor(out=Q[:, 0:wq].rearrange("p (b x) -> p b x", b=n),
                                        in0=tmp[:, 0:wq].rearrange("p (b x) -> p b x", b=n),
                                        in1=odds, op=AL.max)
                R = wp.tile([64, 256], f16, tag="r3", name="r3")
                if d >= 14:
                    nc.vector.tensor_copy(out=R[:, 0:wq], in_=Q[64:128, 0:wq])
                else:
                    nc.gpsimd.tensor_copy(out=R[:, 0:wq], in_=Q[64:128, 0:wq])
                pend3.append((d, Q, R))
                if len(pend3) > 1:
                    l3_final()

            def l3h_direct(dst, Q, R, half):
                nc.vector.tensor_tensor(
                    out=L4B[64:128, dst * 130 + 1:dst * 130 + 129],
                    in0=Q[0:64, half * 128:half * 128 + 128],
                    in1=R[0:64, half * 128:half * 128 + 128], op=AL.max)

            def l3_final():
                d, Q, R = pend3.pop(0)
                v = 2 * d
                if d < 16:
                    nc.vector.tensor_tensor(
                        out=ap_at(L4B, 0, 64, v * 130 + 1, [[130, 2], [1, 128]]),
                        in0=Q[0:64, 0:256].rearrange("p (b x) -> p b x", b=2),
                        in1=R[:, 0:256].rearrange("p (b x) -> p b x", b=2), op=AL.max)
                if d < 16:
                    for dst, srctile in ((2 * d - 1, 2 * d), (2 * d, 2 * d + 1)):
                        if 0 <= dst <= 25:
                            nc.gpsimd.tensor_copy(
                                out=L4B[64:128, dst * 130 + 1:dst * 130 + 129],
                                in_=L4B[0:64, srctile * 130 + 1:srctile * 130 + 129])
                if d == 13:
                    l3h_direct(26, Q, R, 1)
                elif d == 14:
                    l3h_direct(27, Q, R, 0)
                    l3h_direct(28, Q, R, 1)
                elif d == 15:
                    l3h_direct(29, Q, R, 0)
                    l3h_direct(30, Q, R, 1)
                elif d == 16:
                    l3h_direct(31, Q, R, 0)

            def halo34(d0, n):
                nc.sync.dma_start(
                    out=ap_at(L4B, 64, 64, d0 * 130 + 1, [[130, n], [1, 128]]),
                    in_=ap_at(L4B, 0, 64, (d0 + 1) * 130 + 1, [[130, n], [1, 128]]))

            def l4_var(w_):
                return {0: 1, 31: 2}.get(w_, 0)

            def l4_chain(e):
                w_ = 4 * e
                if e >= 6:
                    P4full = pp.tile([128, 1024], f32, tag="ps1", name="ps1", bufs=2)
                    P = P4full[0:128, 0:512]
                else:
                    P4full = pp.tile([128, 512], f32, tag="ps34", name="ps34", bufs=2)
                    P = P4full[:]
                for i in range(4):
                    vb = l4_var(w_ + i) * 384
                    for dx in range(3):
                        nc.tensor.matmul(
                            out=P[:, i * 128:(i + 1) * 128],
                            lhsT=wt4[:, vb + dx * 128:vb + (dx + 1) * 128],
                            rhs=L4B[0:128, (w_ + i) * 130 + dx:(w_ + i) * 130 + dx + 128],
                            start=(dx == 0), stop=(dx == 2))
                stage = wp.tile([128, 512], f32, tag="stage", name="stage")
                nc.scalar.activation(out=stage[:], in_=P[:], func=RELU,
                                     bias=bias4[:], scale=1.0)
                sa = stage[:]
                oa = out_d[:]
                pitch = sa.ap[0][0]
                for rp in range(2):
                    in_ap = AP(sa.tensor, sa.offset + rp * pitch,
                               [[2 * pitch, 64], [128, 4], [1, 128]])
                    out_ap = AP(oa.tensor, 1024 * e + 128 * rp,
                                [[8192, 64], [256, 4], [1, 128]])
                    nc.sync.dma_start(out=out_ap, in_=in_ap)

            # ---- emission schedule ----
            # Rate-limited round-robin: at most one unit per layer per step so
            # the in-order PE queue always has other work between same-layer
            # chains (psum bufs=1), with slack lags so halo-DMA latency is
            # hidden.
            state = dict(l2c=0, h23=0, l3c=0, h34=0, l4c=0)
            stage_ref = [None]
            halo12_cov = [0]
            pend1, pend2, pend3 = [], [], []

            def fin2():
                return state['l2c'] - len(pend2)

            def fin3():
                return state['l3c'] - len(pend3)

            def pump(max_per_layer=1):
                done = 0
                # halo23 DMA chunks m=1..14 (dst [2m-2, 2m)); dst 28-32 are
                # direct TT writes in l2_final
                m = state['h23'] + 1
                if m <= 14 and fin2() >= m + 1:
                    halo23(2 * (m - 1), 2); state['h23'] += 1

                for _ in range(max_per_layer):
                    c = state['l2c']
                    if c < 16 and halo12_cov[0] >= min(2 * c + 2, 33):
                        l2_chain(c); state['l2c'] += 1; done += 1
                    elif c == 16 and halo12_cov[0] >= 33:
                        l2_chain(16); state['l2c'] += 1; done += 1
                if pend2 and state['l2c'] == 17:
                    l2_final()
                for _ in range(max_per_layer):
                    d = state['l3c']
                    if d < 14 and 2 * state['h23'] >= min(2 * d + 6, 28):
                        l3_chain(d); state['l3c'] += 1; done += 1
                    elif d == 14 and fin2() >= 16:
                        l3_chain(14); state['l3c'] += 1; done += 1
                    elif d in (15, 16) and fin2() >= 17:
                        l3_chain(d); state['l3c'] += 1; done += 1
                if pend3 and state['l3c'] == 17:
                    l3_final()
                for _ in range(max_per_layer):
                    e = state['l4c']
                    if e < 6 and fin3() >= min(2 * e + 4, 15):
                        l4_chain(e); state['l4c'] += 1; done += 1
                    elif e == 6 and fin3() >= 15:
                        l4_chain(6); state['l4c'] += 1; done += 1
                    elif e == 7 and fin3() >= 17:
                        l4_chain(7); state['l4c'] += 1; done += 1
                return done

            for t in range(34):
                l1_tile(t)

                pump()
            while pend1:
                l1_final()
            halo12_cov[0] = 33
            for _ in range(64):
                if state['l4c'] == 8:
                    break
                pump()
            assert state['l4c'] == 8, state
            assert not pend1 and not pend2 and not pend3

            if debug:
                nc.sync.dma_start(out=dbg["l2b"][:], in_=L2B[:])
                nc.sync.dma_start(out=dbg["l3b"][:], in_=L3B[:])
                nc.sync.dma_start(out=dbg["l4b"][:], in_=L4B[:])

    nc.finalize()
    return nc


def _prep_weights(inputs):
    params = [
        _fold_weights(inputs['w1'], inputs['b1'], inputs['g1'], inputs['be1'], inputs['m1'], inputs['v1']),
        _fold_weights(inputs['w2'], inputs['b2'], inputs['g2'], inputs['be2'], inputs['m2'], inputs['v2']),
        _fold_weights(inputs['w3'], inputs['b3'], inputs['g3'], inputs['be3'], inputs['m3'], inputs['v3']),
        _fold_weights(inputs['w4'], inputs['b4'], inputs['g4'], inputs['be4'], inputs['m4'], inputs['v4']),
    ]
    wt8 = _build_l1_wt(*params[0])
    # variants per h: [main, v_t0, v_t32, v_t33] etc (zsets depend on h)
    wt2 = {}
    wt3 = {}
    wt4 = {}
    for h in (0, 1):
        if h == 0:
            z2 = [[], [0, 1, 2, 3, 4, 5, 6], [], []]
            z3 = [[], [0, 1, 2], []]
            z4 = [[], [0], []]
        else:
            z2 = [[], [], [7, 8, 9], 'all']
            z3 = [[], [], [3, 4, 5]]
            z4 = [[], [], [3]]
        wt2[h] = _build_wt16(params[1][0], params[1][1], 8, 16, 8, 10, False, True, z2)
        wt3[h] = _build_wt16(params[2][0], params[2][1], 16, 32, 4, 6, False, True, z3)
        wt4[h] = _build_wt16(params[3][0], params[3][1], 32, 64, 2, 4, True, False, z4)
    bias4 = np.zeros((128, 1), np.float32)
    bf4 = params[3][1]
    for o in range(64):
        for y in range(2):
            bias4[o * 2 + y, 0] = bf4[o]
    ones = np.ones((1, 34 * 514), np.float16)
    return wt8, wt2, wt3, wt4, bias4, ones


def kernel(points, batch_size,
           w1, b1, g1, be1, m1, v1,
           w2, b2, g2, be2, m2, v2,
           w3, b3, g3, be3, m3, v3,
           w4, b4, g4, be4, m4, v4, **_kw):
    from concourse.bass_utils import run_bass_kernel_spmd

    grids = _bin_points(points)
    inputs = dict(w1=w1, b1=b1, g1=g1, be1=be1, m1=m1, v1=v1,
                  w2=w2, b2=b2, g2=g2, be2=be2, m2=m2, v2=v2,
                  w3=w3, b3=b3, g3=g3, be3=be3, m3=m3, v3=v3,
                  w4=w4, b4=b4, g4=g4, be4=be4, m4=m4, v4=v4)
    wt8, wt2, wt3, wt4, bias4, ones = _prep_weights(inputs)

    core_ids = list(range(8))
    in_maps = []
    for core in core_ids:
        b, h = core // 2, core % 2
        im = {
            "b8": _build_b8(grids[b], h),
            "wt8": wt8,
            "wt2": wt2[h],
            "wt3": wt3[h],
            "wt4": wt4[h],
            "bias4": bias4,
            "ones": ones,
        }
        in_maps.append(im)

    if "nc" not in _CACHE:
        _CACHE["nc"] = _build_module()
    nc = _CACHE["nc"]

    r = run_bass_kernel_spmd(nc, in_maps, core_ids=core_ids)

    out_full = np.zeros((B, 64, 128, 128), np.float32)
    for i, core in enumerate(core_ids):
        b, h = core // 2, core % 2
        out_full[b, :, 64 * h:64 * h + 64, :] = r.results[i]["out"]
    return out_full


# revision 9
# speedup vs baseline: 1.0933x; 1.0198x over previous
"""BEV histogram + 4x(conv3x3+BN+ReLU) + 3x maxpool on 8 trn2 cores, v2.

Sharding: core = 2*b + h computes output rows [64h, 64h+64) of batch b.
Device pipeline per core (all per-layer activations in one SBUF buffer each):

- L1 in fp8 (e4m3): BEV built+quantized on host as [73, 34*1026] (72 rows =
  18 e-rows x 4 ch + const-1 bias row). Conv via x-pair DoubleRow matmuls:
  even/odd output columns computed separately, 2 fp8 weight blocks (hi + lo
  residual) -> 8 DR matmuls per tile = 2N cycles (vs 3N f16).
- L2-4 f16, 3 matmuls/tile, bias as const-1 K row (L4: bias in ACT epilogue).
- Drain per psum chain: ACT relu-copy-even -> DVE TT max(tmp, psum-odd) ->
  fold-copy Q[64:128] (Pool engine, or DVE in latency-critical phases) ->
  DVE final TT max -> next-layer buffer (f16). relu commutes with max
  (max(relu(a), b) == relu(max(a, b)) since relu(a) >= 0); bias is already
  in psum. Finals are emitted one chain late to hide the fold latency from
  the in-order DVE queue.
- Edges (SAME pad at y-borders): per-core *weight data* variants with the
  out-of-range K-rows zeroed - zero device ops.
- Halos: mid-pipeline via batched SBUF->SBUF DMAs on SP/HWDGE; warmup and
  tail halos via direct partition-shifted TT writes (engine outputs may start
  at any partition; only both-SBUF *inputs* must share a base partition) -
  this removes every DMA hop from the pipeline-drain critical path.
"""
import sys
sys.path.insert(0, '/opt/trn_rl_repo')
import numpy as np
import ml_dtypes

PR = [0.0, -39.68, -3.0, 69.12, 39.68, 1.0]
W = 1024
H = 1024
B = 4
BN_EPS = 1e-5
F8 = ml_dtypes.float8_e4m3

_CACHE = {}


def _bin_points(points):
    pts = np.asarray(points, dtype=np.float32)
    xs = np.float32(W / (PR[3] - PR[0]))
    ys = np.float32(H / (PR[4] - PR[1]))
    half = np.float32((PR[4] - PR[1]) / 2)
    xp = (pts[:, 1] * xs).astype(np.int32)
    yp = ((pts[:, 2] + half) * ys).astype(np.int32)
    b = pts[:, 0].astype(np.int32)
    mask = (xp >= 0) & (xp < W) & (yp >= 0) & (yp < H)
    lin = (b * H + yp) * W + xp
    z = pts[:, 3]
    inten = pts[:, 4]
    n = B * H * W
    lv = lin[mask]
    cnt = np.bincount(lv, minlength=n).astype(np.float32)
    zmin = np.full(n, 10.0, np.float32)
    np.minimum.at(zmin, lv, z[mask])
    zmax = np.full(n, -10.0, np.float32)
    np.maximum.at(zmax, lv, z[mask])
    iv = np.zeros(n, np.float32)
    np.maximum.at(iv, lv, inten[mask])
    bev0 = np.where(cnt == 0, np.float32(1.0), cnt) / np.float32(50.0)
    grids = np.stack([bev0, zmin, zmax, iv], axis=0).reshape(4, B, H, W)
    return np.transpose(grids, (1, 0, 2, 3))


def _fold_weights(w, b, g, be, m, v):
    scale = np.asarray(g, np.float32) / np.sqrt(np.asarray(v, np.float32) + np.float32(BN_EPS))
    wf = np.asarray(w, np.float32) * scale[:, None, None, None]
    bf = (np.asarray(b, np.float32) - np.asarray(m, np.float32)) * scale + np.asarray(be, np.float32)
    return wf.astype(np.float32), bf.astype(np.float32)


def _q8(x):
    return np.asarray(x, np.float32).astype(F8).astype(np.float32)


def _build_l1_wt(wf, bf):
    """-> [73, 7*128] e4m3 blob: blocks [w0h,w1h,w2h,w0l,w1l,w2l,Z].
    m = (y%2)*64 + (y//2)*8 + o; bias (hi/lo) on const row of blocks 0/3."""
    whi = _q8(wf)
    wlo = _q8(wf - whi)
    bhi = _q8(bf)
    blo = _q8(bf - bhi)
    blob = np.zeros((73, 7, 128), np.float32)
    for hl, wq in ((0, whi), (1, wlo)):
        for dx in range(3):
            blk = hl * 3 + dx
            for y in range(16):
                m0 = (y % 2) * 64 + (y // 2) * 8
                for dy in range(3):
                    e = y + dy
                    # rows e*4+c ; cols m0+o
                    blob[e * 4:(e + 1) * 4, blk, m0:m0 + 8] = wq[:, :, dy, dx].T
    for y in range(16):
        m0 = (y % 2) * 64 + (y // 2) * 8
        blob[72, 0, m0:m0 + 8] = bhi
        blob[72, 3, m0:m0 + 8] = blo
    return blob.reshape(73, 7 * 128).astype(F8)


def _m_index(y, o, co, co_major):
    if co_major:
        return o * 2 + y
    return (y % 2) * 64 + (y // 2) * co + o


def _build_wt16(wf, bf, ci, co, yoff, eta, co_major, bias_row, variants):
    """-> [K, nvar*384] f16. variants: list of zsets (e-row lists, or 'all')."""
    K = eta * ci + (1 if bias_row else 0)
    main = np.zeros((K, 3, 128), np.float32)
    for dx in range(3):
        for y in range(yoff):
            for dy in range(3):
                e = y + dy
                for o in range(co):
                    m = _m_index(y, o, co, co_major)
                    main[e * ci:(e + 1) * ci, dx, m] = wf[o, :, dy, dx]
    if bias_row:
        for y in range(yoff):
            for o in range(co):
                main[K - 1, 0, _m_index(y, o, co, co_major)] = bf[o]
    blobs = []
    for zset in variants:
        v = main.copy()
        if zset == 'all':
            v[:] = 0.0
        else:
            for e in zset:
                v[e * ci:(e + 1) * ci] = 0.0
        blobs.append(v)
    out = np.concatenate(blobs, axis=1)  # [K, nvar*3, 128]
    return out.reshape(K, -1).astype(np.float16)


def _build_b8(grid_b, h):
    """grid_b [4, 1024, 1024] f32 -> [73, 34*1026] e4m3 (incl ones row)."""
    from numpy.lib.stride_tricks import sliding_window_view
    g0 = 512 * h - 15
    q = np.asarray(grid_b, np.float32).astype(F8).astype(np.float32)
    padded = np.zeros((4, 546, 1026), np.float32)
    lo = max(0, g0)
    hi = min(1024, g0 + 546)
    padded[:, lo - g0:hi - g0, 1:1025] = q[:, lo:hi, :]
    wins = sliding_window_view(padded, 18, axis=1)    # [4, 529, 1026, 18]
    wins = wins[:, 0:16 * 34:16]                      # [4, 34, 1026, 18]
    tiles = np.transpose(wins, (1, 3, 0, 2))          # [34, 18, 4, 1026]
    tiles = np.ascontiguousarray(tiles).reshape(34, 72, 1026)
    ones = np.ones((34, 1, 1026), np.float32)
    full = np.concatenate([tiles, ones], axis=1)      # [34, 73, 1026]
    return np.ascontiguousarray(np.transpose(full, (1, 0, 2))).reshape(73, 34 * 1026).astype(F8)


def _build_module(debug=False):
    import concourse.mybir as mybir
    from concourse.tile import TileContext
    from concourse import bacc
    from concourse.ap import AP

    f32 = mybir.dt.float32
    f16 = mybir.dt.float16
    f8 = mybir.dt.float8e4
    AL = mybir.AluOpType
    RELU = mybir.ActivationFunctionType.Relu
    DR = mybir.MatmulPerfMode.DoubleRow

    nc = bacc.Bacc()
    b8_d = nc.dram_tensor("b8", [73, 34 * 1026], f8, kind="ExternalInput")
    wt8_d = nc.dram_tensor("wt8", [73, 7 * 128], f8, kind="ExternalInput")
    wt2_d = nc.dram_tensor("wt2", [81, 4 * 384], f16, kind="ExternalInput")
    wt3_d = nc.dram_tensor("wt3", [97, 3 * 384], f16, kind="ExternalInput")
    wt4_d = nc.dram_tensor("wt4", [128, 3 * 384], f16, kind="ExternalInput")
    bias4_d = nc.dram_tensor("bias4", [128, 1], f32, kind="ExternalInput")
    ones_d = nc.dram_tensor("ones", [1, 34 * 514], f16, kind="ExternalInput")
    out_d = nc.dram_tensor("out", [64, 64, 128], f32, kind="ExternalOutput")
    dbg = {}
    if debug:
        dbg["l2b"] = nc.dram_tensor("dbg_l2b", [81, 34 * 514], f16, kind="ExternalOutput")
        dbg["l3b"] = nc.dram_tensor("dbg_l3b", [97, 33 * 258], f16, kind="ExternalOutput")
        dbg["l4b"] = nc.dram_tensor("dbg_l4b", [128, 32 * 130], f16, kind="ExternalOutput")

    def ap3(t, off, pn, d1s, d1n, d2s, d2n):
        a = t[:]
        return AP(a.tensor, a.offset + off, [[a.ap[0][0], pn], [d1s, d1n], [d2s, d2n]])

    def ap_at(t, p0, pn, off, dims):
        a = t[p0:p0 + pn, :]
        return AP(a.tensor, a.offset + off, [[a.ap[0][0], pn]] + dims)

    with TileContext(nc) as tc:
        with tc.tile_pool(name="const", bufs=1) as cp, \
             tc.tile_pool(name="bufs", bufs=1) as bp, \
             tc.tile_pool(name="work", bufs=6) as wp, \
             tc.tile_pool(name="psum", bufs=1, space="PSUM") as pp:

            wt8 = cp.tile([73, 7 * 128], f8, tag="wt8")
            wt2 = cp.tile([81, 4 * 384], f16, tag="wt2")
            wt3 = cp.tile([97, 3 * 384], f16, tag="wt3")
            wt4 = cp.tile([128, 3 * 384], f16, tag="wt4")
            bias4 = cp.tile([128, 1], f32, tag="bias4")
            nc.sync.dma_start(out=wt8[:], in_=wt8_d[:])

            B8 = bp.tile([73, 34 * 1026], f8, tag="B8", name="B8")
            L2B = bp.tile([81, 34 * 514], f16, tag="L2B", name="L2B")
            L3B = bp.tile([97, 33 * 258], f16, tag="L3B", name="L3B")
            L4B = bp.tile([128, 32 * 130], f16, tag="L4B", name="L4B")

            # ones rows (const-1 bias rhs row for L2/L3)
            nc.sync.dma_start(out=L2B[80:81, :], in_=ones_d[:, 0:34 * 514])
            nc.sync.dma_start(out=L3B[96:97, :], in_=ones_d[:, 0:33 * 258])

            # x-pad zero columns + never-written halo of l2 tile 33
            def pad_memsets(buf, pn, ntiles, w_):
                nc.gpsimd.memset(buf[0:pn, 0:1], 0.0)
                nc.gpsimd.memset(ap_at(buf, 0, pn, w_ - 1, [[w_, ntiles - 1], [1, 2]]), 0.0)
                nc.gpsimd.memset(buf[0:pn, ntiles * w_ - 1:ntiles * w_], 0.0)
            pad_memsets(L2B, 80, 34, 514)
            pad_memsets(L3B, 96, 33, 258)
            pad_memsets(L4B, 128, 32, 130)
            nc.gpsimd.memset(L2B[64:80, 33 * 514:34 * 514], 0.0)

            # preload RELU act table while input DMAs are in flight
            warm = cp.tile([1, 2], f16, tag="warm")
            nc.gpsimd.memset(warm[:].bitcast(f32), 0.0)
            nc.scalar.activation(out=warm[:], in_=warm[:], func=RELU, scale=1.0)

            # input chunks: first small for fast start, weights interleaved early
            chunk_bounds = [0, 2, 6, 12, 18, 24, 29, 34]

            def b8_chunk(ci_):
                c0, c1 = chunk_bounds[ci_], chunk_bounds[ci_ + 1]
                nc.sync.dma_start(out=B8[:, c0 * 1026:c1 * 1026],
                                  in_=b8_d[:, c0 * 1026:c1 * 1026])
            b8_chunk(0)
            b8_chunk(1)
            nc.sync.dma_start(out=wt2[:], in_=wt2_d[:])
            b8_chunk(2)
            for t_, d_ in ((wt3, wt3_d), (wt4, wt4_d), (bias4, bias4_d)):
                nc.sync.dma_start(out=t_[:], in_=d_[:])
            for ci_ in range(3, 7):
                b8_chunk(ci_)

            wb8 = wt8[:]

            def lw(i, j):
                return AP(wb8.tensor, wb8.offset + i * 128,
                          [[wb8.ap[0][0], 73], [(j - i) * 128, 2], [1, 128]])

            b8a = B8[:]

            def pairs(off):
                return AP(b8a.tensor, b8a.offset + off, [[b8a.ap[0][0], 73], [1, 2], [2, 256]])

            L1_PAIRS_E = [(lw(0, 1), 0), (lw(3, 4), 0), (lw(2, 6), 2), (lw(5, 6), 2)]
            L1_PAIRS_O = [(lw(6, 0), 0), (lw(6, 3), 0), (lw(1, 2), 2), (lw(4, 5), 2)]

            def l1_tile(t):
                # tiles 0,1 borrow L2's psum slot (L2 starts at t>=5): depth 3
                # in the latency-critical warmup phase
                if t < 2:
                    P = pp.tile([128, 1024], f32, tag="ps2", name="ps2", bufs=1)
                else:
                    P = pp.tile([128, 1024], f32, tag="ps1", name="ps1", bufs=2)
                for reg, plist in ((0, L1_PAIRS_E), (256, L1_PAIRS_O)):
                    for hx in range(2):
                        base = t * 1026 + 512 * hx
                        o0 = hx * 512 + reg
                        for k, (lhs, poff) in enumerate(plist):
                            nc.tensor.matmul(out=P[:, o0:o0 + 256], lhsT=lhs,
                                             rhs=pairs(base + poff),
                                             start=(k == 0), stop=(k == 3),
                                             perf_mode=DR)
                tmp = wp.tile([128, 512], f16, tag="tmp1", name="tmp1")
                Q = wp.tile([128, 512], f16, tag="q1", name="q1")
                Pt = P[:]
                evens = AP(Pt.tensor, Pt.offset, [[Pt.ap[0][0], 128], [512, 2], [1, 256]])
                odds = AP(Pt.tensor, Pt.offset + 256, [[Pt.ap[0][0], 128], [512, 2], [1, 256]])
                nc.scalar.activation(out=tmp[:].rearrange("p (b x) -> p b x", b=2),
                                     in_=evens, func=RELU, scale=1.0)
                nc.vector.tensor_tensor(out=Q[:].rearrange("p (b x) -> p b x", b=2),
                                        in0=tmp[:].rearrange("p (b x) -> p b x", b=2),
                                        in1=odds, op=AL.max)
                R = wp.tile([64, 512], f16, tag="r1", name="r1")
                if t < 24:
                    nc.vector.tensor_copy(out=R[:], in_=Q[64:128, :])
                else:
                    nc.gpsimd.tensor_copy(out=R[:], in_=Q[64:128, :])
                pend1.append((t, Q, R))
                if len(pend1) > 1:
                    l1_final()

            def l1_final():
                t, Q, R = pend1.pop(0)
                nc.vector.tensor_tensor(out=L2B[0:64, t * 514 + 1:t * 514 + 513],
                                        in0=Q[0:64, :], in1=R[:], op=AL.max)
                d = t - 1
                if 0 <= d <= 32:
                    # halo = first 2 pooled rows of tile t (just written):
                    # ACT copy from the main rows instead of a DMA hop
                    nc.scalar.copy(out=L2B[64:80, d * 514 + 1:d * 514 + 513],
                                   in_=L2B[0:16, t * 514 + 1:t * 514 + 513])
                    halo12_cov[0] = max(halo12_cov[0], d + 1)

            def halo12(d0, n):
                nc.sync.dma_start(
                    out=ap_at(L2B, 64, 16, d0 * 514 + 1, [[514, n], [1, 512]]),
                    in_=ap_at(L2B, 0, 16, (d0 + 1) * 514 + 1, [[514, n], [1, 512]]))

            # L2 variant selection: tile -> variant index in wt2 blob
            def l2_var(u):
                return {0: 1, 32: 2, 33: 3}.get(u, 0)

            def l2_chain(c):
                u = 2 * c
                P = pp.tile([128, 1024], f32, tag="ps2", name="ps2", bufs=1)
                for i in range(2):
                    vb = l2_var(u + i) * 384
                    for dx in range(3):
                        nc.tensor.matmul(
                            out=P[:, i * 512:(i + 1) * 512],
                            lhsT=wt2[:, vb + dx * 128:vb + (dx + 1) * 128],
                            rhs=L2B[0:81, (u + i) * 514 + dx:(u + i) * 514 + dx + 512],
                            start=(dx == 0), stop=(dx == 2))
                Pt = P[:]
                evens = AP(Pt.tensor, Pt.offset, [[Pt.ap[0][0], 128], [512, 2], [2, 256]])
                odds = AP(Pt.tensor, Pt.offset + 1, [[Pt.ap[0][0], 128], [512, 2], [2, 256]])
                tmp = wp.tile([128, 512], f16, tag="tmp2", name="tmp2")
                nc.scalar.activation(out=tmp[:].rearrange("p (b x) -> p b x", b=2),
                                     in_=evens, func=RELU, scale=1.0)
                Q = wp.tile([128, 512], f16, tag="q2", name="q2")
                nc.vector.tensor_tensor(out=Q[:].rearrange("p (b x) -> p b x", b=2),
                                        in0=tmp[:].rearrange("p (b x) -> p b x", b=2),
                                        in1=odds, op=AL.max)
                R = wp.tile([64, 512], f16, tag="r2", name="r2")
                if c < 4 or c >= 12:
                    nc.vector.tensor_copy(out=R[:], in_=Q[64:128, :])
                else:
                    nc.gpsimd.tensor_copy(out=R[:], in_=Q[64:128, :])
                pend2.append((c, Q, R))
                if len(pend2) > 1:
                    l2_final()

            def l2h_direct(dst, Q, R, half):
                # dst tile's halo rows (parts 64:96) directly from this final's
                # first 2 pooled rows (parts 0:32) - no DMA hop
                nc.vector.tensor_tensor(
                    out=L3B[64:96, dst * 258 + 1:dst * 258 + 257],
                    in0=Q[0:32, half * 256:half * 256 + 256],
                    in1=R[0:32, half * 256:half * 256 + 256], op=AL.max)

            def l2_final():
                c, Q, R = pend2.pop(0)
                u = 2 * c
                if c < 16:
                    nc.vector.tensor_tensor(
                        out=ap_at(L3B, 0, 64, u * 258 + 1, [[258, 2], [1, 256]]),
                        in0=Q[0:64, :].rearrange("p (b x) -> p b x", b=2),
                        in1=R[:].rearrange("p (b x) -> p b x", b=2), op=AL.max)
                else:
                    nc.vector.tensor_tensor(out=L3B[0:64, 32 * 258 + 1:32 * 258 + 257],
                                            in0=Q[0:64, 0:256], in1=R[:, 0:256], op=AL.max)
                if c >= 14:
                    for dst in (2 * c, 2 * c + 1):
                        if 28 <= dst <= 31:
                            nc.scalar.copy(
                                out=L3B[64:96, dst * 258 + 1:dst * 258 + 257],
                                in_=L3B[0:32, (dst + 1) * 258 + 1:(dst + 1) * 258 + 257])
                if c == 16:
                    l2h_direct(32, Q, R, 1)

            def halo23(d0, n):
                nc.sync.dma_start(
                    out=ap_at(L3B, 64, 32, d0 * 258 + 1, [[258, n], [1, 256]]),
                    in_=ap_at(L3B, 0, 32, (d0 + 1) * 258 + 1, [[258, n], [1, 256]]))

            def l3_var(v):
                return {0: 1, 32: 2}.get(v, 0)

            def l3_chain(d):
                v = 2 * d
                single = (d == 16)
                n = 1 if single else 2
                if d >= 14:
                    Pf = pp.tile([128, 1024], f32, tag="ps2", name="ps2", bufs=1)
                    P = Pf[0:128, 0:512]
                else:
                    P = pp.tile([128, 512], f32, tag="ps34", name="ps34", bufs=2)
                for i in range(n):
                    vb = l3_var(v + i) * 384
                    for dx in range(3):
                        nc.tensor.matmul(
                            out=P[:, i * 256:(i + 1) * 256],
                            lhsT=wt3[:, vb + dx * 128:vb + (dx + 1) * 128],
                            rhs=L3B[0:97, (v + i) * 258 + dx:(v + i) * 258 + dx + 256],
                            start=(dx == 0), stop=(dx == 2))
                Pt = P[:]
                wq = 128 * n
                evens = AP(Pt.tensor, Pt.offset, [[Pt.ap[0][0], 128], [256, n], [2, 128]])
                odds = AP(Pt.tensor, Pt.offset + 1, [[Pt.ap[0][0], 128], [256, n], [2, 128]])
                tmp = wp.tile([128, 256], f16, tag="tmp3", name="tmp3")
                nc.scalar.activation(out=tmp[:, 0:wq].rearrange("p (b x) -> p b x", b=n),
                                     in_=evens, func=RELU, scale=1.0)
                Q = wp.tile([128, 256], f16, tag="q3", name="q3")
                nc.vector.tensor_tensor(out=Q[:, 0:wq].rearrange("p (b x) -> p b x", b=n),
                                        in0=tmp[:, 0:wq].rearrange("p (b x) -> p b x", b=n),
                                        in1=odds, op=AL.max)
                R = wp.tile([64, 256], f16, tag="r3", name="r3")
                if d >= 14:
                    nc.vector.tensor_copy(out=R[:, 0:wq], in_=Q[64:128, 0:wq])
                else:
                    nc.gpsimd.tensor_copy(out=R[:, 0:wq], in_=Q[64:128, 0:wq])
                pend3.append((d, Q, R))
                if len(pend3) > 1:
                    l3_final()

            def l3h_direct(dst, Q, R, half):
                nc.vector.tensor_tensor(
                    out=L4B[64:128, dst * 130 + 1:dst * 130 + 129],
                    in0=Q[0:64, half * 128:half * 128 + 128],
                    in1=R[0:64, half * 128:half * 128 + 128], op=AL.max)

            def l3_final():
                d, Q, R = pend3.pop(0)
                v = 2 * d
                if d < 16:
                    nc.vector.tensor_tensor(
                        out=ap_at(L4B, 0, 64, v * 130 + 1, [[130, 2], [1, 128]]),
                        in0=Q[0:64, 0:256].rearrange("p (b x) -> p b x", b=2),
                        in1=R[:, 0:256].rearrange("p (b x) -> p b x", b=2), op=AL.max)
                if d < 16:
                    for dst, srctile in ((2 * d - 1, 2 * d), (2 * d, 2 * d + 1)):
                        if 0 <= dst <= 30:
                            nc.gpsimd.tensor_copy(
                                out=L4B[64:128, dst * 130 + 1:dst * 130 + 129],
                                in_=L4B[0:64, srctile * 130 + 1:srctile * 130 + 129])
                if d == 16:
                    l3h_direct(31, Q, R, 0)

            def halo34(d0, n):
                nc.sync.dma_start(
                    out=ap_at(L4B, 64, 64, d0 * 130 + 1, [[130, n], [1, 128]]),
                    in_=ap_at(L4B, 0, 64, (d0 + 1) * 130 + 1, [[130, n], [1, 128]]))

            def l4_var(w_):
                return {0: 1, 31: 2}.get(w_, 0)

            def l4_chain(e):
                w_ = 4 * e
                if e >= 6:
                    P4full = pp.tile([128, 1024], f32, tag="ps1", name="ps1", bufs=2)
                    P = P4full[0:128, 0:512]
                else:
                    P4full = pp.tile([128, 512], f32, tag="ps34", name="ps34", bufs=2)
                    P = P4full[:]
                for i in range(4):
                    vb = l4_var(w_ + i) * 384
                    for dx in range(3):
                        nc.tensor.matmul(
                            out=P[:, i * 128:(i + 1) * 128],
                            lhsT=wt4[:, vb + dx * 128:vb + (dx + 1) * 128],
                            rhs=L4B[0:128, (w_ + i) * 130 + dx:(w_ + i) * 130 + dx + 128],
                            start=(dx == 0), stop=(dx == 2))
                stage = wp.tile([128, 512], f32, tag="stage", name="stage")
                nc.scalar.activation(out=stage[:], in_=P[:], func=RELU,
                                     bias=bias4[:], scale=1.0)
                sa = stage[:]
                oa = out_d[:]
                pitch = sa.ap[0][0]
                for rp in range(2):
                    in_ap = AP(sa.tensor, sa.offset + rp * pitch,
                               [[2 * pitch, 64], [128, 4], [1, 128]])
                    out_ap = AP(oa.tensor, 1024 * e + 128 * rp,
                                [[8192, 64], [256, 4], [1, 128]])
                    nc.sync.dma_start(out=out_ap, in_=in_ap)

            # ---- emission schedule ----
            # Rate-limited round-robin: at most one unit per layer per step so
            # the in-order PE queue always has other work between same-layer
            # chains (psum bufs=1), with slack lags so halo-DMA latency is
            # hidden.
            state = dict(l2c=0, h23=0, l3c=0, h34=0, l4c=0)
            stage_ref = [None]
            halo12_cov = [0]
            pend1, pend2, pend3 = [], [], []

            def fin2():
                return state['l2c'] - len(pend2)

            def fin3():
                return state['l3c'] - len(pend3)

            def pump(max_per_layer=1):
                done = 0
                # halo23 DMA chunks m=1..14 (dst [2m-2, 2m)); dst 28-32 are
                # direct TT writes in l2_final
                m = state['h23'] + 1
                if m <= 14 and fin2() >= m + 1:
                    halo23(2 * (m - 1), 2); state['h23'] += 1

                for _ in range(max_per_layer):
                    c = state['l2c']
                    if c < 16 and halo12_cov[0] >= min(2 * c + 2, 33):
                        l2_chain(c); state['l2c'] += 1; done += 1
                    elif c == 16 and halo12_cov[0] >= 33:
                        l2_chain(16); state['l2c'] += 1; done += 1
                if pend2 and state['l2c'] == 17:
                    l2_final()
                for _ in range(max_per_layer):
                    d = state['l3c']
                    if d < 14 and 2 * state['h23'] >= min(2 * d + 6, 28):
                        l3_chain(d); state['l3c'] += 1; done += 1
                    elif d == 14 and fin2() >= 16:
                        l3_chain(14); state['l3c'] += 1; done += 1
                    elif d in (15, 16) and fin2() >= 17:
                        l3_chain(d); state['l3c'] += 1; done += 1
                if pend3 and state['l3c'] == 17:
                    l3_final()
                for _ in range(max_per_layer):
                    e = state['l4c']
                    if e < 6 and fin3() >= min(2 * e + 4, 15):
                        l4_chain(e); state['l4c'] += 1; done += 1
                    elif e == 6 and fin3() >= 15:
                        l4_chain(6); state['l4c'] += 1; done += 1
                    elif e == 7 and fin3() >= 17:
                        l4_chain(7); state['l4c'] += 1; done += 1
                return done

            for t in range(34):
                l1_tile(t)

                pump()
            while pend1:
                l1_final()
            halo12_cov[0] = 33
            for _ in range(64):
                if state['l4c'] == 8:
                    break
                pump()
            assert state['l4c'] == 8, state
            assert not pend1 and not pend2 and not pend3

            if debug:
                nc.sync.dma_start(out=dbg["l2b"][:], in_=L2B[:])
                nc.sync.dma_start(out=dbg["l3b"][:], in_=L3B[:])
                nc.sync.dma_start(out=dbg["l4b"][:], in_=L4B[:])

    nc.finalize()
    return nc


def _prep_weights(inputs):
    params = [
        _fold_weights(inputs['w1'], inputs['b1'], inputs['g1'], inputs['be1'], inputs['m1'], inputs['v1']),
        _fold_weights(inputs['w2'], inputs['b2'], inputs['g2'], inputs['be2'], inputs['m2'], inputs['v2']),
        _fold_weights(inputs['w3'], inputs['b3'], inputs['g3'], inputs['be3'], inputs['m3'], inputs['v3']),
        _fold_weights(inputs['w4'], inputs['b4'], inputs['g4'], inputs['be4'], inputs['m4'], inputs['v4']),
    ]
    wt8 = _build_l1_wt(*params[0])
    # variants per h: [main, v_t0, v_t32, v_t33] etc (zsets depend on h)
    wt2 = {}
    wt3 = {}
    wt4 = {}
    for h in (0, 1):
        if h == 0:
            z2 = [[], [0, 1, 2, 3, 4, 5, 6], [], []]
            z3 = [[], [0, 1, 2], []]
            z4 = [[], [0], []]
        else:
            z2 = [[], [], [7, 8, 9], 'all']
            z3 = [[], [], [3, 4, 5]]
            z4 = [[], [], [3]]
        wt2[h] = _build_wt16(params[1][0], params[1][1], 8, 16, 8, 10, False, True, z2)
        wt3[h] = _build_wt16(params[2][0], params[2][1], 16, 32, 4, 6, False, True, z3)
        wt4[h] = _build_wt16(params[3][0], params[3][1], 32, 64, 2, 4, True, False, z4)
    bias4 = np.zeros((128, 1), np.float32)
    bf4 = params[3][1]
    for o in range(64):
        for y in range(2):
            bias4[o * 2 + y, 0] = bf4[o]
    ones = np.ones((1, 34 * 514), np.float16)
    return wt8, wt2, wt3, wt4, bias4, ones


def kernel(points, batch_size,
           w1, b1, g1, be1, m1, v1,
           w2, b2, g2, be2, m2, v2,
           w3, b3, g3, be3, m3, v3,
           w4, b4, g4, be4, m4, v4, **_kw):
    from concourse.bass_utils import run_bass_kernel_spmd

    grids = _bin_points(points)
    inputs = dict(w1=w1, b1=b1, g1=g1, be1=be1, m1=m1, v1=v1,
                  w2=w2, b2=b2, g2=g2, be2=be2, m2=m2, v2=v2,
                  w3=w3, b3=b3, g3=g3, be3=be3, m3=m3, v3=v3,
                  w4=w4, b4=b4, g4=g4, be4=be4, m4=m4, v4=v4)
    wt8, wt2, wt3, wt4, bias4, ones = _prep_weights(inputs)

    core_ids = list(range(8))
    in_maps = []
    for core in core_ids:
        b, h = core // 2, core % 2
        im = {
            "b8": _build_b8(grids[b], h),
            "wt8": wt8,
            "wt2": wt2[h],
            "wt3": wt3[h],
            "wt4": wt4[h],
            "bias4": bias4,
            "ones": ones,
        }
        in_maps.append(im)

    if "nc" not in _CACHE:
        _CACHE["nc"] = _build_module()
    nc = _CACHE["nc"]

    r = run_bass_kernel_spmd(nc, in_maps, core_ids=core_ids)

    out_full = np.zeros((B, 64, 128, 128), np.float32)
    for i, core in enumerate(core_ids):
        b, h = core // 2, core % 2
        out_full[b, :, 64 * h:64 * h + 64, :] = r.results[i]["out"]
    return out_full
